# revision 11
# baseline (speedup 1.0000x reference)
"""Longformer encoder (12-layer, sliding-window attention) on 8 Trainium2 cores.

Sharding: (batch=4) x (seq half=2) -> 8 cores; 1024 tokens/core.
Sliding-window attention (+-256) uses a per-layer K/V halo exchange between
the two cores of each batch pair via a 2-rank AllGather.

On-device layout is feature-major: activations [feature_partition, token].
  - projections:  psum[outf, tok] = W[inf, outf].T @ h[inf, tok]   (W stationary)
  - V:            psum[tok, outf] = h[inf, tok].T @ Wv[inf, outf]  (h stationary)
  - scores:       psum[ktok, qtok] = K[hd, ktok].T @ Q[hd, qtok]
  - PV:           psum[hd(+1), qtok] = Vtok[ktok, hd+1].T @ P[ktok, qtok]
                  (extra all-ones column of Vtok yields the softmax denominator)
All matmuls bf16 with fp32 PSUM accumulation; layernorm/softmax math fp32.
"""

import os
import numpy as np
import ml_dtypes

B, S, C = 4, 2048, 128
H, NH, HD, FF, W1 = 768, 12, 64, 3072, 256
L = int(os.environ.get("KERNEL_NL", "12"))
T = 1024            # tokens per core
KP = T + 2 * W1     # padded key range per core (1536)
HT = H // 128       # feature tiles (6)
FT = FF // 128      # ffn feature tiles (24)
VH = HD + 1         # v columns per head incl ones column (65)
VW = NH * VH        # v row width per token tile (780)
EPS = 1e-5
NEG = -30000.0
ISQ = float(1.0 / np.sqrt(HD))

bf16 = ml_dtypes.bfloat16

_CACHED = {}


def _build(n_layers):
    import concourse.bacc as bacc
    import concourse.mybir as mybir
    from concourse import tile
    from contextlib import ExitStack

    dt = mybir.dt
    AF = mybir.ActivationFunctionType
    OP = mybir.AluOpType

    nc = bacc.Bacc(None, target_bir_lowering=False, debug=False)

    # ---------------- DRAM I/O ----------------
    xT = nc.dram_tensor("xT", [C, T], dt.bfloat16, kind="ExternalInput")
    pe = nc.dram_tensor("pe", [H, T], dt.float32, kind="ExternalInput")
    mks = nc.dram_tensor("mks", [16, 128, 512], dt.bfloat16, kind="ExternalInput")

    up_w1 = nc.dram_tensor("up_w1", [C, H], dt.bfloat16, kind="ExternalInput")
    up_w2 = nc.dram_tensor("up_w2", [H, H], dt.bfloat16, kind="ExternalInput")
    dn_w1 = nc.dram_tensor("dn_w1", [H, H], dt.bfloat16, kind="ExternalInput")
    dn_w2 = nc.dram_tensor("dn_w2", [H, C], dt.bfloat16, kind="ExternalInput")

    Wq = nc.dram_tensor("Wq", [n_layers, H, H], dt.bfloat16, kind="ExternalInput")
    Wk = nc.dram_tensor("Wk", [n_layers, H, H], dt.bfloat16, kind="ExternalInput")
    Wv = nc.dram_tensor("Wv", [n_layers, H, H], dt.bfloat16, kind="ExternalInput")
    Wo = nc.dram_tensor("Wo", [n_layers, H, H], dt.bfloat16, kind="ExternalInput")
    Wi = nc.dram_tensor("Wi", [n_layers, H, FF], dt.bfloat16, kind="ExternalInput")
    Wo2 = nc.dram_tensor("Wo2", [n_layers, FF, H], dt.bfloat16, kind="ExternalInput")

    # per-feature vectors, host-packed as [128, ntiles] (column j = feats 128j:128j+128)
    # order: bq bk bv bo g1 b1 bo2 g2 b2 pad
    vecs = nc.dram_tensor("vecs", [n_layers, 128, 10 * HT], dt.float32, kind="ExternalInput")
    bi_all = nc.dram_tensor("bi_all", [n_layers, 128, FT], dt.float32, kind="ExternalInput")
    # order: up_b1 up_b2 ln_g ln_b dn_b1 pad
    gvec = nc.dram_tensor("gvec", [128, 6 * HT], dt.float32, kind="ExternalInput")
    dn_b2 = nc.dram_tensor("dn_b2", [128, 1], dt.float32, kind="ExternalInput")

    y = nc.dram_tensor("y", [C, T], dt.float32, kind="ExternalOutput")

    # halo exchange buffers (reused across layers)
    # send: [k_left(6*128*256) k_right | v_left(2*128*768) v_right] bf16
    KSEG = HT * 128 * W1
    VSEG = 2 * 128 * H
    SEND_N = 2 * KSEG + 2 * VSEG
    cc_send = nc.dram_tensor("cc_send", [SEND_N], dt.bfloat16)
    cc_recv = nc.dram_tensor("cc_recv", [2, SEND_N], dt.bfloat16)
    K_OFF = [0, KSEG]
    V_OFF = [2 * KSEG, 2 * KSEG + VSEG]

    with tile.TileContext(nc) as tc, ExitStack() as ctx:
        pp = ctx.enter_context(tc.tile_pool(name="persist", bufs=1))
        wp = ctx.enter_context(tc.tile_pool(name="w768", bufs=6))
        bp = ctx.enter_context(tc.tile_pool(name="bias", bufs=2))
        fp = ctx.enter_context(tc.tile_pool(name="ffmid", bufs=2))
        sp = ctx.enter_context(tc.tile_pool(name="scratch", bufs=1))
        rp = ctx.enter_context(tc.tile_pool(name="rows", bufs=2))
        pbp = ctx.enter_context(tc.tile_pool(name="pbuf", bufs=2))
        psA = ctx.enter_context(tc.tile_pool(name="psA", bufs=2, space="PSUM"))

        # ---------------- persistent tiles ----------------
        h = pp.tile([128, HT * T], dt.float32, tag="h")          # residual stream
        hb = pp.tile([128, HT * T], dt.bfloat16, tag="hb")       # bf16 copy of stream
        qb = pp.tile([128, HT * T], dt.bfloat16, tag="qb")       # Q (feature-major)
        ob = pp.tile([128, HT * T], dt.bfloat16, tag="ob")       # attn out (feature-major)
        # K padded, feature-major, split [left pad | interior | right pad]
        Kpl = pp.tile([128, HT * W1], dt.bfloat16, tag="Kpl")
        Kpm = pp.tile([128, HT * T], dt.bfloat16, tag="Kpm")
        Kpr = pp.tile([128, HT * W1], dt.bfloat16, tag="Kpr")
        # V padded, token-major (65-wide head slots), split by token tiles 0-1|2-9|10-11
        Vpl = pp.tile([128, 2 * VW], dt.bfloat16, tag="Vpl")
        Vpm = pp.tile([128, 8 * VW], dt.bfloat16, tag="Vpm")
        Vpr = pp.tile([128, 2 * VW], dt.bfloat16, tag="Vpr")
        msk = pp.tile([128, 16 * 512], dt.bfloat16, tag="msk")   # additive masks
        ones_c = pp.tile([128, 1], dt.bfloat16, tag="ones_c")    # [128,1] ones (stats lhsT)
        ones_r = pp.tile([128, 128], dt.bfloat16, tag="ones_r")  # ones (bcast lhsT slices)
        gv = pp.tile([128, 6 * HT], dt.float32, tag="gv")
        dnb2 = pp.tile([128, 1], dt.float32, tag="dnb2")

        def hs(ft, qt=None):
            if qt is None:
                return slice(ft * T, (ft + 1) * T)
            return slice(ft * T + qt * 512, ft * T + qt * 512 + 512)

        def k_ap(ft, kt):  # lhsT AP [128, 128] for padded key tile kt (0..11)
            if kt < 2:
                return Kpl[:, ft * W1 + kt * 128: ft * W1 + (kt + 1) * 128]
            if kt < 10:
                return Kpm[:, ft * T + (kt - 2) * 128: ft * T + (kt - 1) * 128]
            return Kpr[:, ft * W1 + (kt - 10) * 128: ft * W1 + (kt - 9) * 128]

        def v_ap(i):  # full [128, VW] token tile i of padded V (0..11)
            if i < 2:
                return Vpl[:, i * VW:(i + 1) * VW]
            if i < 10:
                return Vpm[:, (i - 2) * VW:(i - 1) * VW]
            return Vpr[:, (i - 10) * VW:(i - 9) * VW]

        nc.vector.memset(ones_c[:], 1.0)
        nc.vector.memset(ones_r[:], 1.0)
        for i in range(12):
            vre = v_ap(i).rearrange("p (n c) -> p n c", c=VH)
            nc.vector.memset(vre[:, :, HD:HD + 1], 1.0)

        nc.sync.dma_start(out=gv[:], in_=gvec[:])
        nc.sync.dma_start(out=dnb2[:], in_=dn_b2[:])
        for j in range(16):
            nc.sync.dma_start(out=msk[:, j * 512:(j + 1) * 512], in_=mks[j])

        # ---------------- helpers ----------------
        def w_proj(w_dram, rhs_cols, out_fn, n_k=HT):
            """psum[mt][qt] = W.T @ rhs ; out_fn(mt, qt, psum) evicts."""
            wt = []
            for k in range(n_k):
                t = wp.tile([128, H], dt.bfloat16, tag="w768")
                nc.sync.dma_start(out=t[:], in_=w_dram[k * 128:(k + 1) * 128, :])
                wt.append(t)
            for mt in range(HT):
                for qt in range(2):
                    pt = psA.tile([128, 512], dt.float32, tag="work")
                    for k in range(n_k):
                        nc.tensor.matmul(
                            pt[:], wt[k][:, mt * 128:(mt + 1) * 128],
                            rhs_cols(k, qt),
                            start=(k == 0), stop=(k == n_k - 1))
                    out_fn(mt, qt, pt)

        # layernorm: reads/writes h in place, writes hb
        def layernorm(g_col, b_col, lp):
            ub = sp.tile([128, HT * T], dt.bfloat16, tag="scr")
            for ft in range(HT):
                nc.scalar.activation(ub[:, hs(ft)], h[:, hs(ft)], AF.Copy)
            for qt in range(2):
                mp = lp.tile([1, 512], dt.float32, tag="statm")
                for ft in range(HT):
                    nc.tensor.matmul(mp[:], ones_c[:], ub[:, hs(ft, qt)],
                                     start=(ft == 0), stop=(ft == HT - 1))
                mrow = rp.tile([1, 512], dt.bfloat16, tag="mrow")
                nc.scalar.activation(mrow[:], mp[:], AF.Copy, scale=1.0 / H)
                m2 = rp.tile([1, 512], dt.float32, tag="rowA")
                nc.scalar.activation(m2[:], mp[:], AF.Square, scale=1.0 / H)
                for ft in range(HT):
                    nc.scalar.square(ub[:, hs(ft, qt)], ub[:, hs(ft, qt)])
                sq = lp.tile([1, 512], dt.float32, tag="stats")
                for ft in range(HT):
                    nc.tensor.matmul(sq[:], ones_c[:], ub[:, hs(ft, qt)],
                                     start=(ft == 0), stop=(ft == HT - 1))
                var = rp.tile([1, 512], dt.float32, tag="rowB")
                nc.vector.scalar_tensor_tensor(var[:], sq[:], 1.0 / H, m2[:],
                                               OP.mult, OP.subtract)
                nc.vector.tensor_scalar_add(var[:], var[:], EPS)
                rec = rp.tile([1, 512], dt.float32, tag="rowA", name="rec")
                nc.vector.reciprocal(rec[:], var[:])
                rrow = rp.tile([1, 512], dt.bfloat16, tag="mrow", name="rrow")
                nc.scalar.activation(rrow[:], rec[:], AF.Sqrt)
                mb = lp.tile([128, 512], dt.float32, tag="mb")
                nc.tensor.matmul(mb[:], ones_r[0:1, :], mrow[:], start=True, stop=True)
                rb = lp.tile([128, 512], dt.float32, tag="rb")
                nc.tensor.matmul(rb[:], ones_r[0:1, :], rrow[:], start=True, stop=True)
                for ft in range(HT):
                    sl = hs(ft, qt)
                    nc.vector.tensor_sub(h[:, sl], h[:, sl], mb[:])
                    nc.vector.scalar_tensor_tensor(h[:, sl], h[:, sl],
                                                   g_col(ft), rb[:], OP.mult, OP.mult)
                    nc.vector.tensor_scalar_add(h[:, sl], h[:, sl], b_col(ft))
                    nc.scalar.activation(hb[:, sl], h[:, sl], AF.Copy)

        # ---------------- input projection ----------------
        # h <- pos_emb + t_emb  (host-combined), then += up-proj
        for ft in range(HT):
            nc.sync.dma_start(out=h[:, hs(ft)], in_=pe[ft * 128:(ft + 1) * 128, :])
        xb = sp.tile([128, T], dt.bfloat16, tag="xb")
        nc.sync.dma_start(out=xb[:], in_=xT[:])

        w1t = wp.tile([128, H], dt.bfloat16, tag="w768")
        nc.sync.dma_start(out=w1t[:], in_=up_w1[:])
        t1 = sp.tile([128, HT * T], dt.bfloat16, tag="scr")
        for mt in range(HT):
            for qt in range(2):
                pt = psA.tile([128, 512], dt.float32, tag="work")
                nc.tensor.matmul(pt[:], w1t[:, mt * 128:(mt + 1) * 128],
                                 xb[:, qt * 512:qt * 512 + 512], start=True, stop=True)
                nc.scalar.activation(t1[:, hs(mt, qt)], pt[:], AF.Tanh,
                                     bias=gv[:, 0 * HT + mt:0 * HT + mt + 1])

        def up2_out(mt, qt, pt):
            sl = hs(mt, qt)
            nc.vector.scalar_tensor_tensor(
                h[:, sl], pt[:], gv[:, 1 * HT + mt:1 * HT + mt + 1], h[:, sl],
                OP.add, OP.add)
        w_proj(up_w2, lambda k, qt: t1[:, hs(k, qt)], up2_out)

        with tc.tile_pool(name="lnps", bufs=1, space="PSUM") as lp0:
            layernorm(lambda ft: gv[:, 2 * HT + ft:2 * HT + ft + 1],
                      lambda ft: gv[:, 3 * HT + ft:3 * HT + ft + 1], lp0)

        # ---------------- encoder layers ----------------
        for l in range(n_layers):
            bv_t = bp.tile([128, 10 * HT], dt.float32, tag="bvec")
            nc.sync.dma_start(out=bv_t[:], in_=vecs[l])
            bi_t = bp.tile([128, FT], dt.float32, tag="bivec")
            nc.sync.dma_start(out=bi_t[:], in_=bi_all[l])

            def vcol(i, ft):
                return bv_t[:, i * HT + ft:i * HT + ft + 1]

            # --- Q, K projections (feature-major) ---
            def q_out(mt, qt, pt):
                nc.scalar.activation(qb[:, hs(mt, qt)], pt[:], AF.Identity,
                                     bias=vcol(0, mt))
            w_proj(Wq[l], lambda k, qt: hb[:, hs(k, qt)], q_out)

            def k_out(mt, qt, pt):
                sl = slice(mt * T + qt * 512, mt * T + qt * 512 + 512)
                nc.scalar.activation(Kpm[:, sl], pt[:], AF.Identity, bias=vcol(1, mt))
            w_proj(Wk[l], lambda k, qt: hb[:, hs(k, qt)], k_out)

            # --- V projection (token-major: h stationary) ---
            wvt = []
            for k in range(HT):
                t = wp.tile([128, H], dt.bfloat16, tag="w768")
                nc.sync.dma_start(out=t[:], in_=Wv[l, k * 128:(k + 1) * 128, :])
                wvt.append(t)
            for tt in range(T // 128):
                for n0, nn in ((0, 512), (512, 256)):
                    pt = psA.tile([128, 512], dt.float32, tag="work")
                    for k in range(HT):
                        nc.tensor.matmul(
                            pt[:, :nn],
                            hb[:, k * T + tt * 128:k * T + tt * 128 + 128],
                            wvt[k][:, n0:n0 + nn],
                            start=(k == 0), stop=(k == HT - 1))
                    dst = v_ap(2 + tt).rearrange("p (n c) -> p n c", c=VH)
                    h0, nh_ = n0 // HD, nn // HD
                    src = pt[:, :nn].rearrange("p (n c) -> p n c", c=HD)
                    nc.scalar.activation(dst[:, h0:h0 + nh_, 0:HD], src[:], AF.Copy)

            # --- halo exchange ---
            for ft in range(HT):
                nc.sync.dma_start(
                    out=cc_send[K_OFF[0] + ft * 128 * W1:K_OFF[0] + (ft + 1) * 128 * W1]
                    .rearrange("(p t) -> p t", p=128),
                    in_=Kpm[:, ft * T:ft * T + W1])
                nc.sync.dma_start(
                    out=cc_send[K_OFF[1] + ft * 128 * W1:K_OFF[1] + (ft + 1) * 128 * W1]
                    .rearrange("(p t) -> p t", p=128),
                    in_=Kpm[:, ft * T + T - W1:(ft + 1) * T])
            for i in range(2):
                for side, tt in ((0, 2 + i), (1, 8 + i)):
                    off = V_OFF[side] + i * 128 * H
                    nc.sync.dma_start(
                        out=cc_send[off:off + 128 * H]
                        .rearrange("(p n c) -> p n c", p=128, c=HD),
                        in_=v_ap(tt).rearrange("p (n c) -> p n c", c=VH)[:, :, 0:HD])
            nc.gpsimd.collective_compute(
                "AllGather", OP.bypass, ins=[cc_send[:]], outs=[cc_recv[:]],
                replica_groups=[[0, 1], [2, 3], [4, 5], [6, 7]])
            # my left halo <- rank0's right edge ; my right halo <- rank1's left edge
            for ft in range(HT):
                nc.sync.dma_start(
                    out=Kpl[:, ft * W1:(ft + 1) * W1],
                    in_=cc_recv[0, K_OFF[1] + ft * 128 * W1:K_OFF[1] + (ft + 1) * 128 * W1]
                    .rearrange("(p t) -> p t", p=128))
                nc.sync.dma_start(
                    out=Kpr[:, ft * W1:(ft + 1) * W1],
                    in_=cc_recv[1, K_OFF[0] + ft * 128 * W1:K_OFF[0] + (ft + 1) * 128 * W1]
                    .rearrange("(p t) -> p t", p=128))
            for i in range(2):
                for rank, src_side, tt in ((0, 1, i), (1, 0, 10 + i)):
                    off = V_OFF[src_side] + i * 128 * H
                    nc.sync.dma_start(
                        out=v_ap(tt).rearrange("p (n c) -> p n c", c=VH)[:, :, 0:HD],
                        in_=cc_recv[rank, off:off + 128 * H]
                        .rearrange("(p n c) -> p n c", p=128, c=HD))

            # --- attention per head ---
            att_cm = tc.tile_pool(name="attps", bufs=2, space="PSUM")
            att_ps = att_cm.__enter__()
            for hd_i in range(NH):
                ft, r0 = hd_i // 2, (hd_i % 2) * HD
                for qt in range(2):
                    pb = pbp.tile([128, 8 * 512], dt.bfloat16, tag="pb")
                    for j in range(8):
                        kt = 4 * qt + j
                        sc = psA.tile([128, 512], dt.float32, tag="work")
                        nc.tensor.matmul(
                            sc[:], k_ap(ft, kt)[r0:r0 + HD, :],
                            qb[r0:r0 + HD, hs(ft, qt)],
                            start=True, stop=True)
                        pj = pb[:, j * 512:(j + 1) * 512]
                        nc.vector.tensor_add(
                            pj, sc[:], msk[:, (qt * 8 + j) * 512:(qt * 8 + j + 1) * 512])
                        nc.scalar.activation(pj, pj, AF.Exp, scale=ISQ)
                    pv = att_ps.tile([VH, 512], dt.float32, tag="pv")
                    for j in range(8):
                        nc.tensor.matmul(
                            pv[:], v_ap(4 * qt + j)[:, hd_i * VH:(hd_i + 1) * VH],
                            pb[:, j * 512:(j + 1) * 512],
                            start=(j == 0), stop=(j == 7))
                    r128 = rp.tile([128, 512], dt.float32, tag="r128")
                    rb128 = rp.tile([128, 512], dt.bfloat16, tag="rb128")
                    nc.vector.reciprocal(r128[HD:VH, :], pv[HD:VH, :])
                    nc.vector.tensor_copy(rb128[HD:VH, :], r128[HD:VH, :])
                    bc = att_ps.tile([HD, 512], dt.float32, tag="bcr")
                    nc.tensor.matmul(bc[:], ones_r[HD:VH, 0:HD], rb128[HD:VH, :],
                                     start=True, stop=True)
                    bcs = rp.tile([HD, 512], dt.bfloat16, tag="otmp", name="bcs")
                    nc.scalar.activation(bcs[:], bc[:], AF.Copy)
                    ot = rp.tile([HD, 512], dt.bfloat16, tag="otmp")
                    nc.vector.tensor_mul(ot[:], pv[0:HD, :], bcs[:])
                    nc.scalar.activation(
                        ob[r0:r0 + HD, hs(ft, qt)], ot[:], AF.Identity,
                        bias=vcol(2, ft)[r0:r0 + HD, :])

            att_cm.__exit__(None, None, None)

            # --- O projection + residual + LN1 ---
            def o_out(mt, qt, pt):
                sl = hs(mt, qt)
                nc.vector.scalar_tensor_tensor(h[:, sl], pt[:], vcol(3, mt), h[:, sl],
                                               OP.add, OP.add)
            w_proj(Wo[l], lambda k, qt: ob[:, hs(k, qt)], o_out)
            with tc.tile_pool(name="lnps", bufs=1, space="PSUM") as lp1:
                layernorm(lambda ft: vcol(4, ft), lambda ft: vcol(5, ft), lp1)

            # --- FFN ---
            ffn_cm = tc.tile_pool(name="ffps", bufs=1, space="PSUM")
            ffn_ps = ffn_cm.__enter__()
            for qt in range(2):
                fpt = [ffn_ps.tile([128, 512], dt.float32, tag=f"ff2_{m}", name=f"ff2_{m}") for m in range(HT)]
                for ch in range(6):  # 6 chunks of 4 mid tiles (512 cols of FF)
                    wi_ch = []
                    for k in range(HT):
                        t = wp.tile([128, 512], dt.bfloat16, tag="wi")
                        nc.sync.dma_start(
                            out=t[:],
                            in_=Wi[l, k * 128:(k + 1) * 128, ch * 512:(ch + 1) * 512])
                        wi_ch.append(t)
                    fmid = fp.tile([128, 4 * 512], dt.bfloat16, tag="fmid")
                    for mi in range(4):
                        mt = ch * 4 + mi
                        pt = psA.tile([128, 512], dt.float32, tag="work")
                        for k in range(HT):
                            nc.tensor.matmul(
                                pt[:], wi_ch[k][:, mi * 128:(mi + 1) * 128],
                                hb[:, hs(k, qt)],
                                start=(k == 0), stop=(k == HT - 1))
                        nc.scalar.activation(fmid[:, mi * 512:(mi + 1) * 512], pt[:],
                                             AF.Gelu, bias=bi_t[:, mt:mt + 1])
                    wo2_ch = []
                    for mi in range(4):
                        t = wp.tile([128, H], dt.bfloat16, tag="w768")
                        nc.sync.dma_start(
                            out=t[:],
                            in_=Wo2[l, (ch * 4 + mi) * 128:(ch * 4 + mi + 1) * 128, :])
                        wo2_ch.append(t)
                    for m in range(HT):
                        for mi in range(4):
                            kt = ch * 4 + mi
                            nc.tensor.matmul(
                                fpt[m][:], wo2_ch[mi][:, m * 128:(m + 1) * 128],
                                fmid[:, mi * 512:(mi + 1) * 512],
                                start=(kt == 0), stop=(kt == FT - 1))
                for m in range(HT):
                    sl = hs(m, qt)
                    nc.vector.scalar_tensor_tensor(h[:, sl], fpt[m][:], vcol(6, m),
                                                   h[:, sl], OP.add, OP.add)
            ffn_cm.__exit__(None, None, None)
            with tc.tile_pool(name="lnps", bufs=1, space="PSUM") as lp2:
                layernorm(lambda ft: vcol(7, ft), lambda ft: vcol(8, ft), lp2)

        # ---------------- output projection ----------------
        t2 = sp.tile([128, HT * T], dt.bfloat16, tag="scr")

        def d1_out(mt, qt, pt):
            nc.scalar.activation(t2[:, hs(mt, qt)], pt[:], AF.Tanh,
                                 bias=gv[:, 4 * HT + mt:4 * HT + mt + 1])
        w_proj(dn_w1, lambda k, qt: hb[:, hs(k, qt)], d1_out)

        w2t = wp.tile([128, HT * C], dt.bfloat16, tag="w768")
        for k in range(HT):
            nc.sync.dma_start(out=w2t[:, k * C:(k + 1) * C],
                              in_=dn_w2[k * 128:(k + 1) * 128, :])
        for qt in range(2):
            pt = psA.tile([128, 512], dt.float32, tag="work")
            for k in range(HT):
                nc.tensor.matmul(pt[:], w2t[:, k * C:(k + 1) * C],
                                 t2[:, hs(k, qt)], start=(k == 0), stop=(k == HT - 1))
            yo = rp.tile([128, 512], dt.float32, tag="r128", name="yout")
            nc.scalar.activation(yo[:], pt[:], AF.Identity, bias=dnb2[:])
            nc.sync.dma_start(out=y[:, qt * 512:qt * 512 + 512], in_=yo[:])

    nc.compile()
    return nc


def _host_prep(inputs, n_layers):
    f32 = np.float32
    x = np.asarray(inputs["x"], f32)
    ts = np.asarray(inputs["timesteps"])
    half = C // 2
    freqs = np.exp(-np.log(10000.0) * np.arange(half, dtype=f32) / half)
    a = ts.astype(f32)[:, None] * freqs[None, :]
    emb0 = np.concatenate([np.cos(a), np.sin(a)], -1).astype(f32)
    t1 = emb0 @ np.asarray(inputs["t_w1"], f32) + np.asarray(inputs["t_b1"], f32)
    t1 = t1 / (1.0 + np.exp(-t1))
    emb = (t1 @ np.asarray(inputs["t_w2"], f32) + np.asarray(inputs["t_b2"], f32)).astype(f32)

    def cvt(w):
        return np.ascontiguousarray(np.asarray(w, f32).astype(bf16))

    def packvec(v, nt):
        return np.ascontiguousarray(np.asarray(v, f32).reshape(nt, 128).T)

    com = dict(
        up_w1=cvt(inputs["up_w1"]), up_w2=cvt(inputs["up_w2"]),
        dn_w1=cvt(inputs["down_w1"]), dn_w2=cvt(inputs["down_w2"]),
        Wq=cvt(inputs["Wq"][:n_layers]), Wk=cvt(inputs["Wk"][:n_layers]),
        Wv=cvt(inputs["Wv"][:n_layers]), Wo=cvt(inputs["Wo"][:n_layers]),
        Wi=cvt(inputs["Wi"][:n_layers]), Wo2=cvt(inputs["Wo2"][:n_layers]),
        dn_b2=np.ascontiguousarray(np.asarray(inputs["down_b2"], f32).reshape(1, C).T),
    )
    vecs = np.stack([
        np.concatenate([packvec(np.asarray(inputs[k], f32)[l], HT) for k in
                        ("bq", "bk", "bv", "bo", "g1", "b1", "bo2", "g2", "b2", "b2")],
                       axis=1)
        for l in range(n_layers)])
    com["vecs"] = np.ascontiguousarray(vecs.astype(f32))
    com["bi_all"] = np.ascontiguousarray(
        np.stack([packvec(np.asarray(inputs["bi"], f32)[l], FT)
                  for l in range(n_layers)]).astype(f32))
    com["gvec"] = np.ascontiguousarray(np.concatenate([
        packvec(inputs["up_b1"], HT), packvec(inputs["up_b2"], HT),
        packvec(inputs["ln_g"], HT), packvec(inputs["ln_b"], HT),
        packvec(inputs["down_b1"], HT), packvec(inputs["down_b1"], HT)],
        axis=1).astype(f32))

    pos = np.asarray(inputs["pos_emb"], f32)
    mk = {}
    for sh in range(2):
        base = sh * T
        m = np.empty((2, 8, 128, 512), f32)
        for qt in range(2):
            for j in range(8):
                gk = base - W1 + (4 * qt + j) * 128 + np.arange(128)[:, None]
                gq = base + qt * 512 + np.arange(512)[None, :]
                valid = (np.abs(gk - gq) <= W1) & (gk >= 0) & (gk < S)
                m[qt, j] = np.where(valid, 0.0, NEG)
        mk[sh] = np.ascontiguousarray(m.reshape(16, 128, 512).astype(bf16))

    in_maps = []
    for c in range(8):
        b, sh = c // 2, c % 2
        sl = slice(sh * T, (sh + 1) * T)
        im = dict(com)
        im["xT"] = np.ascontiguousarray(x[b, sl].T.astype(bf16))
        im["pe"] = np.ascontiguousarray((pos[sl] + emb[b][None, :]).T.astype(f32))
        im["mks"] = mk[sh]
        in_maps.append(im)
    return in_maps


def kernel(**inputs):
    from concourse.bass_utils import run_bass_kernel_spmd

    n_layers = L
    if n_layers not in _CACHED:
        _CACHED[n_layers] = _build(n_layers)
    nc = _CACHED[n_layers]
    in_maps = _host_prep(inputs, n_layers)
    trace = os.environ.get("KERNEL_TRACE", "0") == "1"
    tmpdir = os.environ.get("KERNEL_TMPDIR") or None
    res = run_bass_kernel_spmd(nc, in_maps, list(range(8)), trace=trace,
                               tmpdir=tmpdir)
    if getattr(res, "exec_time_ns", None):
        print(f"HW exec time: {res.exec_time_ns} ns")
    out = np.empty((B, S, C), np.float32)
    for c in range(8):
        b, sh = c // 2, c % 2
        out[b, sh * T:(sh + 1) * T, :] = res.results[c]["y"].T
    return out


# revision 12
# speedup vs baseline: 1.1794x; 1.1794x over previous
"""Longformer encoder (12-layer, sliding-window attention) on 8 Trainium2 cores.

Sharding: (batch=4) x (seq half=2) -> 8 cores; 1024 tokens/core.
Sliding-window attention (+-256) uses a per-layer K/V halo exchange between
the two cores of each batch pair via a 2-rank AllGather.

On-device layout is feature-major: activations [feature_partition, token].
  - projections:  psum[outf, tok] = W[inf, outf].T @ h[inf, tok]   (W stationary)
  - V:            psum[tok, outf] = h[inf, tok].T @ Wv[inf, outf]  (h stationary)
  - scores:       psum[ktok, qtok] = K[hd, ktok].T @ Q[hd, qtok]
  - PV:           psum[hd(+1), qtok] = Vtok[ktok, hd+1].T @ P[ktok, qtok]
                  (extra all-ones column of Vtok yields the softmax denominator)
All matmuls bf16 with fp32 PSUM accumulation; layernorm/softmax math fp32.
"""

import os
import numpy as np
import ml_dtypes

B, S, C = 4, 2048, 128
H, NH, HD, FF, W1 = 768, 12, 64, 3072, 256
L = int(os.environ.get("KERNEL_NL", "12"))
T = 1024            # tokens per core
KP = T + 2 * W1     # padded key range per core (1536)
HT = H // 128       # feature tiles (6)
FT = FF // 128      # ffn feature tiles (24)
VH = HD + 1         # v columns per head incl ones column (65)
VW = NH * VH        # v row width per token tile (780)
EPS = 1e-5
NEG = -30000.0
ISQ = float(1.0 / np.sqrt(HD))

bf16 = ml_dtypes.bfloat16

_CACHED = {}


def _build(n_layers):
    import concourse.bacc as bacc
    import concourse.mybir as mybir
    from concourse import tile
    from contextlib import ExitStack

    dt = mybir.dt
    AF = mybir.ActivationFunctionType
    OP = mybir.AluOpType

    nc = bacc.Bacc(None, target_bir_lowering=False, debug=False)

    # ---------------- DRAM I/O ----------------
    xT = nc.dram_tensor("xT", [C, T], dt.bfloat16, kind="ExternalInput")
    pe = nc.dram_tensor("pe", [H, T], dt.float32, kind="ExternalInput")
    mks = nc.dram_tensor("mks", [16, 128, 512], dt.bfloat16, kind="ExternalInput")

    up_w1 = nc.dram_tensor("up_w1", [C, H], dt.bfloat16, kind="ExternalInput")
    up_w2 = nc.dram_tensor("up_w2", [H, H], dt.bfloat16, kind="ExternalInput")
    dn_w1 = nc.dram_tensor("dn_w1", [H, H], dt.bfloat16, kind="ExternalInput")
    dn_w2 = nc.dram_tensor("dn_w2", [H, C], dt.bfloat16, kind="ExternalInput")

    Wq = nc.dram_tensor("Wq", [n_layers, H, H], dt.bfloat16, kind="ExternalInput")
    Wk = nc.dram_tensor("Wk", [n_layers, H, H], dt.bfloat16, kind="ExternalInput")
    Wv = nc.dram_tensor("Wv", [n_layers, H, H], dt.bfloat16, kind="ExternalInput")
    Wo = nc.dram_tensor("Wo", [n_layers, H, H], dt.bfloat16, kind="ExternalInput")
    Wi = nc.dram_tensor("Wi", [n_layers, H, FF], dt.bfloat16, kind="ExternalInput")
    Wo2 = nc.dram_tensor("Wo2", [n_layers, FF, H], dt.bfloat16, kind="ExternalInput")

    # per-feature vectors, host-packed as [128, ntiles] (column j = feats 128j:128j+128)
    # order: bq bk bv bo g1 b1 bo2 g2 b2 pad
    vecs = nc.dram_tensor("vecs", [n_layers, 128, 10 * HT], dt.float32, kind="ExternalInput")
    bi_all = nc.dram_tensor("bi_all", [n_layers, 128, FT], dt.float32, kind="ExternalInput")
    # order: up_b1 up_b2 ln_g ln_b dn_b1 pad
    gvec = nc.dram_tensor("gvec", [128, 6 * HT], dt.float32, kind="ExternalInput")
    dn_b2 = nc.dram_tensor("dn_b2", [128, 1], dt.float32, kind="ExternalInput")

    y = nc.dram_tensor("y", [C, T], dt.float32, kind="ExternalOutput")

    # halo exchange buffers: h-stream edges (halo K/V projected locally)
    KSEG = HT * 128 * W1
    SEND_N = 2 * KSEG
    cc_send = nc.dram_tensor("cc_send", [SEND_N], dt.bfloat16)
    cc_recv = nc.dram_tensor("cc_recv", [2, SEND_N], dt.bfloat16)
    K_OFF = [0, KSEG]

    with tile.TileContext(nc) as tc, ExitStack() as ctx:
        pp = ctx.enter_context(tc.tile_pool(name="persist", bufs=1))
        wp = ctx.enter_context(tc.tile_pool(name="w768", bufs=6))
        bp = ctx.enter_context(tc.tile_pool(name="bias", bufs=2))
        fp = ctx.enter_context(tc.tile_pool(name="ffmid", bufs=2))
        sp = ctx.enter_context(tc.tile_pool(name="scratch", bufs=1))
        rp = ctx.enter_context(tc.tile_pool(name="rows", bufs=2))
        pbp = ctx.enter_context(tc.tile_pool(name="pbuf", bufs=3))
        psA = ctx.enter_context(tc.tile_pool(name="psA", bufs=2, space="PSUM"))

        # ---------------- persistent tiles ----------------
        h = pp.tile([128, HT * T], dt.float32, tag="h")          # residual stream
        hb = pp.tile([128, HT * T], dt.bfloat16, tag="hb")       # bf16 copy of stream
        qb = pp.tile([128, HT * T], dt.bfloat16, tag="qb")       # Q (feature-major)
        ob = pp.tile([128, HT * T], dt.bfloat16, tag="ob")       # attn out (feature-major)
        # K padded, feature-major, split [left pad | interior | right pad]
        Kpl = pp.tile([128, HT * W1], dt.bfloat16, tag="Kpl")
        Kpm = pp.tile([128, HT * T], dt.bfloat16, tag="Kpm")
        Kpr = pp.tile([128, HT * W1], dt.bfloat16, tag="Kpr")
        # V padded, token-major (65-wide head slots), split by token tiles 0-1|2-9|10-11
        Vpl = pp.tile([128, 2 * VW], dt.bfloat16, tag="Vpl")
        Vpm = pp.tile([128, 8 * VW], dt.bfloat16, tag="Vpm")
        Vpr = pp.tile([128, 2 * VW], dt.bfloat16, tag="Vpr")
        hbh = pp.tile([128, HT * 512], dt.bfloat16, tag="hbh")   # halo h [left|right]
        msk = pp.tile([128, 16 * 512], dt.bfloat16, tag="msk")   # additive masks
        ones_c = pp.tile([128, 1], dt.bfloat16, tag="ones_c")    # [128,1] ones (stats lhsT)
        ones_r = pp.tile([128, 128], dt.bfloat16, tag="ones_r")  # ones (bcast lhsT slices)
        gv = pp.tile([128, 6 * HT], dt.float32, tag="gv")
        dnb2 = pp.tile([128, 1], dt.float32, tag="dnb2")

        def hs(ft, qt=None):
            if qt is None:
                return slice(ft * T, (ft + 1) * T)
            return slice(ft * T + qt * 512, ft * T + qt * 512 + 512)

        def k_ap(ft, kt):  # lhsT AP [128, 128] for padded key tile kt (0..11)
            if kt < 2:
                return Kpl[:, ft * W1 + kt * 128: ft * W1 + (kt + 1) * 128]
            if kt < 10:
                return Kpm[:, ft * T + (kt - 2) * 128: ft * T + (kt - 1) * 128]
            return Kpr[:, ft * W1 + (kt - 10) * 128: ft * W1 + (kt - 9) * 128]

        def v_ap(i):  # full [128, VW] token tile i of padded V (0..11)
            if i < 2:
                return Vpl[:, i * VW:(i + 1) * VW]
            if i < 10:
                return Vpm[:, (i - 2) * VW:(i - 1) * VW]
            return Vpr[:, (i - 10) * VW:(i - 9) * VW]

        nc.vector.memset(ones_c[:], 1.0)
        nc.vector.memset(ones_r[:], 1.0)
        for i in range(12):
            vre = v_ap(i).rearrange("p (n c) -> p n c", c=VH)
            nc.vector.memset(vre[:, :, HD:HD + 1], 1.0)

        nc.sync.dma_start(out=gv[:], in_=gvec[:])
        nc.sync.dma_start(out=dnb2[:], in_=dn_b2[:])
        for j in range(16):
            nc.sync.dma_start(out=msk[:, j * 512:(j + 1) * 512], in_=mks[j])

        # ---------------- helpers ----------------
        def w_proj(w_dram, rhs_cols, out_fn, n_k=HT, nq=2):
            """psum[mt][qt] = W.T @ rhs ; out_fn(mt, qt, psum) evicts."""
            wt = []
            for k in range(n_k):
                t = wp.tile([128, H], dt.bfloat16, tag="w768")
                nc.sync.dma_start(out=t[:], in_=w_dram[k * 128:(k + 1) * 128, :])
                wt.append(t)
            for mt in range(HT):
                for qt in range(nq):
                    pt = psA.tile([128, 512], dt.float32, tag="work")
                    for k in range(n_k):
                        nc.tensor.matmul(
                            pt[:], wt[k][:, mt * 128:(mt + 1) * 128],
                            rhs_cols(k, qt),
                            start=(k == 0), stop=(k == n_k - 1))
                    out_fn(mt, qt, pt)

        # layernorm: reads/writes h in place, writes hb
        def layernorm(g_col, b_col, lp):
            ub = sp.tile([128, HT * T], dt.bfloat16, tag="scr")
            for ft in range(HT):
                nc.scalar.activation(ub[:, hs(ft)], h[:, hs(ft)], AF.Copy)
            for qt in range(2):
                mp = lp.tile([1, 512], dt.float32, tag="statm")
                for ft in range(HT):
                    nc.tensor.matmul(mp[:], ones_c[:], ub[:, hs(ft, qt)],
                                     start=(ft == 0), stop=(ft == HT - 1))
                mrow = rp.tile([1, 512], dt.bfloat16, tag="mrow")
                nc.scalar.activation(mrow[:], mp[:], AF.Copy, scale=1.0 / H)
                m2 = rp.tile([1, 512], dt.float32, tag="rowA")
                nc.scalar.activation(m2[:], mp[:], AF.Square, scale=1.0 / H)
                for ft in range(HT):
                    nc.scalar.square(ub[:, hs(ft, qt)], ub[:, hs(ft, qt)])
                sq = lp.tile([1, 512], dt.float32, tag="stats")
                for ft in range(HT):
                    nc.tensor.matmul(sq[:], ones_c[:], ub[:, hs(ft, qt)],
                                     start=(ft == 0), stop=(ft == HT - 1))
                var = rp.tile([1, 512], dt.float32, tag="rowB")
                nc.vector.scalar_tensor_tensor(var[:], sq[:], 1.0 / H, m2[:],
                                               OP.mult, OP.subtract)
                nc.vector.tensor_scalar_add(var[:], var[:], EPS)
                rec = rp.tile([1, 512], dt.float32, tag="rowA", name="rec")
                nc.scalar.activation(rec[:], var[:], AF.Ln)
                rrow = rp.tile([1, 512], dt.bfloat16, tag="mrow", name="rrow")
                nc.scalar.activation(rrow[:], rec[:], AF.Exp, scale=-0.5)
                mb = lp.tile([128, 512], dt.float32, tag="mb")
                nc.tensor.matmul(mb[:], ones_r[0:1, :], mrow[:], start=True, stop=True)
                rb = lp.tile([128, 512], dt.float32, tag="rb")
                nc.tensor.matmul(rb[:], ones_r[0:1, :], rrow[:], start=True, stop=True)
                for ft in range(HT):
                    sl = hs(ft, qt)
                    nc.vector.tensor_sub(h[:, sl], h[:, sl], mb[:])
                    nc.vector.scalar_tensor_tensor(h[:, sl], h[:, sl],
                                                   g_col(ft), rb[:], OP.mult, OP.mult)
                    nc.vector.tensor_scalar_add(h[:, sl], h[:, sl], b_col(ft))
                    nc.scalar.activation(hb[:, sl], h[:, sl], AF.Copy)

        # ---------------- input projection ----------------
        # h <- pos_emb + t_emb  (host-combined), then += up-proj
        for ft in range(HT):
            nc.sync.dma_start(out=h[:, hs(ft)], in_=pe[ft * 128:(ft + 1) * 128, :])
        xb = sp.tile([128, T], dt.bfloat16, tag="xb")
        nc.sync.dma_start(out=xb[:], in_=xT[:])

        w1t = wp.tile([128, H], dt.bfloat16, tag="w768")
        nc.sync.dma_start(out=w1t[:], in_=up_w1[:])
        t1 = sp.tile([128, HT * T], dt.bfloat16, tag="scr")
        for mt in range(HT):
            for qt in range(2):
                pt = psA.tile([128, 512], dt.float32, tag="work")
                nc.tensor.matmul(pt[:], w1t[:, mt * 128:(mt + 1) * 128],
                                 xb[:, qt * 512:qt * 512 + 512], start=True, stop=True)
                nc.scalar.activation(t1[:, hs(mt, qt)], pt[:], AF.Tanh,
                                     bias=gv[:, 0 * HT + mt:0 * HT + mt + 1])

        def up2_out(mt, qt, pt):
            sl = hs(mt, qt)
            nc.vector.scalar_tensor_tensor(
                h[:, sl], pt[:], gv[:, 1 * HT + mt:1 * HT + mt + 1], h[:, sl],
                OP.add, OP.add)
        w_proj(up_w2, lambda k, qt: t1[:, hs(k, qt)], up2_out)

        with tc.tile_pool(name="lnps", bufs=1, space="PSUM") as lp0:
            layernorm(lambda ft: gv[:, 2 * HT + ft:2 * HT + ft + 1],
                      lambda ft: gv[:, 3 * HT + ft:3 * HT + ft + 1], lp0)

        # ---------------- encoder layers ----------------
        for l in range(n_layers):
            bv_t = bp.tile([128, 10 * HT], dt.float32, tag="bvec")
            nc.sync.dma_start(out=bv_t[:], in_=vecs[l])
            bi_t = bp.tile([128, FT], dt.float32, tag="bivec")
            nc.sync.dma_start(out=bi_t[:], in_=bi_all[l])

            def vcol(i, ft):
                return bv_t[:, i * HT + ft:i * HT + ft + 1]

            # --- h-edge halo exchange (overlaps with Q/K/V projections) ---
            for ft in range(HT):
                nc.sync.dma_start(
                    out=cc_send[K_OFF[0] + ft * 128 * W1:K_OFF[0] + (ft + 1) * 128 * W1]
                    .rearrange("(p t) -> p t", p=128),
                    in_=hb[:, ft * T:ft * T + W1])
                nc.sync.dma_start(
                    out=cc_send[K_OFF[1] + ft * 128 * W1:K_OFF[1] + (ft + 1) * 128 * W1]
                    .rearrange("(p t) -> p t", p=128),
                    in_=hb[:, ft * T + T - W1:(ft + 1) * T])
            nc.gpsimd.collective_compute(
                "AllGather", OP.bypass, ins=[cc_send[:]], outs=[cc_recv[:]],
                replica_groups=[[0, 1], [2, 3], [4, 5], [6, 7]])
            # my left halo <- rank0's right edge ; my right halo <- rank1's left edge
            for ft in range(HT):
                nc.sync.dma_start(
                    out=hbh[:, ft * 512:ft * 512 + W1],
                    in_=cc_recv[0, K_OFF[1] + ft * 128 * W1:K_OFF[1] + (ft + 1) * 128 * W1]
                    .rearrange("(p t) -> p t", p=128))
                nc.sync.dma_start(
                    out=hbh[:, ft * 512 + W1:(ft + 1) * 512],
                    in_=cc_recv[1, K_OFF[0] + ft * 128 * W1:K_OFF[0] + (ft + 1) * 128 * W1]
                    .rearrange("(p t) -> p t", p=128))

            # --- Q, K projections (feature-major) ---
            def q_out(mt, qt, pt):
                nc.scalar.activation(qb[:, hs(mt, qt)], pt[:], AF.Identity,
                                     bias=vcol(0, mt))
            w_proj(Wq[l], lambda k, qt: hb[:, hs(k, qt)], q_out)

            def k_out(mt, qt, pt):
                if qt < 2:
                    sl = slice(mt * T + qt * 512, mt * T + qt * 512 + 512)
                    nc.scalar.activation(Kpm[:, sl], pt[:], AF.Identity, bias=vcol(1, mt))
                else:
                    nc.scalar.activation(Kpl[:, mt * W1:(mt + 1) * W1], pt[:, 0:W1],
                                         AF.Identity, bias=vcol(1, mt))
                    nc.scalar.activation(Kpr[:, mt * W1:(mt + 1) * W1], pt[:, W1:512],
                                         AF.Identity, bias=vcol(1, mt))

            def k_rhs(k, qt):
                if qt < 2:
                    return hb[:, hs(k, qt)]
                return hbh[:, k * 512:(k + 1) * 512]
            w_proj(Wk[l], k_rhs, k_out, nq=3)

            # --- V projection (token-major: h stationary) ---
            wvt = []
            for k in range(HT):
                t = wp.tile([128, H], dt.bfloat16, tag="w768")
                nc.sync.dma_start(out=t[:], in_=Wv[l, k * 128:(k + 1) * 128, :])
                wvt.append(t)
            def v_stat(vt, k):  # stationary h slice for padded v token tile vt
                if vt < 2:
                    return hbh[:, k * 512 + vt * 128:k * 512 + (vt + 1) * 128]
                if vt < 10:
                    tt = vt - 2
                    return hb[:, k * T + tt * 128:k * T + tt * 128 + 128]
                return hbh[:, k * 512 + W1 + (vt - 10) * 128:k * 512 + W1 + (vt - 9) * 128]

            for vt in range(12):
                for n0, nn in ((0, 512), (512, 256)):
                    pt = psA.tile([128, 512], dt.float32, tag="work")
                    for k in range(HT):
                        nc.tensor.matmul(
                            pt[:, :nn], v_stat(vt, k), wvt[k][:, n0:n0 + nn],
                            start=(k == 0), stop=(k == HT - 1))
                    dst = v_ap(vt).rearrange("p (n c) -> p n c", c=VH)
                    h0, nh_ = n0 // HD, nn // HD
                    src = pt[:, :nn].rearrange("p (n c) -> p n c", c=HD)
                    nc.scalar.activation(dst[:, h0:h0 + nh_, 0:HD], src[:], AF.Copy)

            # --- attention per head ---
            att_cm = tc.tile_pool(name="attps", bufs=2, space="PSUM")
            att_ps = att_cm.__enter__()
            for hd_i in range(NH):
                ft, r0 = hd_i // 2, (hd_i % 2) * HD
                for qt in range(2):
                    pb = pbp.tile([128, 8 * 512], dt.bfloat16, tag="pb")
                    for j in range(8):
                        kt = 4 * qt + j
                        sc = psA.tile([128, 512], dt.float32, tag="work")
                        nc.tensor.matmul(
                            sc[:], k_ap(ft, kt)[r0:r0 + HD, :],
                            qb[r0:r0 + HD, hs(ft, qt)],
                            start=True, stop=True)
                        pj = pb[:, j * 512:(j + 1) * 512]
                        nc.vector.tensor_add(
                            pj, sc[:], msk[:, (qt * 8 + j) * 512:(qt * 8 + j + 1) * 512])
                    nc.scalar.activation(pb[:], pb[:], AF.Exp, scale=ISQ)
                    pv = att_ps.tile([VH, 512], dt.float32, tag="pv")
                    for j in range(8):
                        nc.tensor.matmul(
                            pv[:], v_ap(4 * qt + j)[:, hd_i * VH:(hd_i + 1) * VH],
                            pb[:, j * 512:(j + 1) * 512],
                            start=(j == 0), stop=(j == 7))
                    r128 = rp.tile([128, 512], dt.float32, tag="r128")
                    rb128 = rp.tile([128, 512], dt.bfloat16, tag="rb128")
                    nc.scalar.activation(r128[HD:VH, :], pv[HD:VH, :], AF.Ln)
                    nc.scalar.activation(rb128[HD:VH, :], r128[HD:VH, :], AF.Exp,
                                         scale=-1.0)
                    bc = att_ps.tile([HD, 512], dt.float32, tag="bcr")
                    nc.tensor.matmul(bc[:], ones_r[HD:VH, 0:HD], rb128[HD:VH, :],
                                     start=True, stop=True)
                    bcs = rp.tile([HD, 512], dt.bfloat16, tag="otmp", name="bcs")
                    nc.scalar.activation(bcs[:], bc[:], AF.Copy)
                    ot = rp.tile([HD, 512], dt.bfloat16, tag="otmp")
                    nc.vector.tensor_mul(ot[:], pv[0:HD, :], bcs[:])
                    nc.scalar.activation(
                        ob[r0:r0 + HD, hs(ft, qt)], ot[:], AF.Identity,
                        bias=vcol(2, ft)[r0:r0 + HD, :])

            att_cm.__exit__(None, None, None)

            # --- O projection + residual + LN1 ---
            def o_out(mt, qt, pt):
                sl = hs(mt, qt)
                nc.vector.scalar_tensor_tensor(h[:, sl], pt[:], vcol(3, mt), h[:, sl],
                                               OP.add, OP.add)
            w_proj(Wo[l], lambda k, qt: ob[:, hs(k, qt)], o_out)
            with tc.tile_pool(name="lnps", bufs=1, space="PSUM") as lp1:
                layernorm(lambda ft: vcol(4, ft), lambda ft: vcol(5, ft), lp1)

            # --- FFN ---
            ffn_cm = tc.tile_pool(name="ffps", bufs=1, space="PSUM")
            ffn_ps = ffn_cm.__enter__()
            for qt in range(2):
                fpt = [ffn_ps.tile([128, 512], dt.float32, tag=f"ff2_{m}", name=f"ff2_{m}") for m in range(HT)]
                for ch in range(6):  # 6 chunks of 4 mid tiles (512 cols of FF)
                    wi_ch = []
                    for k in range(HT):
                        t = wp.tile([128, 512], dt.bfloat16, tag="wi")
                        nc.sync.dma_start(
                            out=t[:],
                            in_=Wi[l, k * 128:(k + 1) * 128, ch * 512:(ch + 1) * 512])
                        wi_ch.append(t)
                    fmid = fp.tile([128, 4 * 512], dt.bfloat16, tag="fmid")
                    for mi in range(4):
                        mt = ch * 4 + mi
                        pt = psA.tile([128, 512], dt.float32, tag="work")
                        for k in range(HT):
                            nc.tensor.matmul(
                                pt[:], wi_ch[k][:, mi * 128:(mi + 1) * 128],
                                hb[:, hs(k, qt)],
                                start=(k == 0), stop=(k == HT - 1))
                        nc.scalar.activation(fmid[:, mi * 512:(mi + 1) * 512], pt[:],
                                             AF.Gelu, bias=bi_t[:, mt:mt + 1])
                    wo2_ch = []
                    for mi in range(4):
                        t = wp.tile([128, H], dt.bfloat16, tag="w768")
                        nc.sync.dma_start(
                            out=t[:],
                            in_=Wo2[l, (ch * 4 + mi) * 128:(ch * 4 + mi + 1) * 128, :])
                        wo2_ch.append(t)
                    for m in range(HT):
                        for mi in range(4):
                            kt = ch * 4 + mi
                            nc.tensor.matmul(
                                fpt[m][:], wo2_ch[mi][:, m * 128:(m + 1) * 128],
                                fmid[:, mi * 512:(mi + 1) * 512],
                                start=(kt == 0), stop=(kt == FT - 1))
                for m in range(HT):
                    sl = hs(m, qt)
                    nc.vector.scalar_tensor_tensor(h[:, sl], fpt[m][:], vcol(6, m),
                                                   h[:, sl], OP.add, OP.add)
            ffn_cm.__exit__(None, None, None)
            with tc.tile_pool(name="lnps", bufs=1, space="PSUM") as lp2:
                layernorm(lambda ft: vcol(7, ft), lambda ft: vcol(8, ft), lp2)

        # ---------------- output projection ----------------
        t2 = sp.tile([128, HT * T], dt.bfloat16, tag="scr")

        def d1_out(mt, qt, pt):
            nc.scalar.activation(t2[:, hs(mt, qt)], pt[:], AF.Tanh,
                                 bias=gv[:, 4 * HT + mt:4 * HT + mt + 1])
        w_proj(dn_w1, lambda k, qt: hb[:, hs(k, qt)], d1_out)

        w2t = wp.tile([128, HT * C], dt.bfloat16, tag="w768")
        for k in range(HT):
            nc.sync.dma_start(out=w2t[:, k * C:(k + 1) * C],
                              in_=dn_w2[k * 128:(k + 1) * 128, :])
        for qt in range(2):
            pt = psA.tile([128, 512], dt.float32, tag="work")
            for k in range(HT):
                nc.tensor.matmul(pt[:], w2t[:, k * C:(k + 1) * C],
                                 t2[:, hs(k, qt)], start=(k == 0), stop=(k == HT - 1))
            yo = rp.tile([128, 512], dt.float32, tag="r128", name="yout")
            nc.scalar.activation(yo[:], pt[:], AF.Identity, bias=dnb2[:])
            nc.sync.dma_start(out=y[:, qt * 512:qt * 512 + 512], in_=yo[:])

    nc.compile()
    return nc


def _host_prep(inputs, n_layers):
    f32 = np.float32
    x = np.asarray(inputs["x"], f32)
    ts = np.asarray(inputs["timesteps"])
    half = C // 2
    freqs = np.exp(-np.log(10000.0) * np.arange(half, dtype=f32) / half)
    a = ts.astype(f32)[:, None] * freqs[None, :]
    emb0 = np.concatenate([np.cos(a), np.sin(a)], -1).astype(f32)
    t1 = emb0 @ np.asarray(inputs["t_w1"], f32) + np.asarray(inputs["t_b1"], f32)
    t1 = t1 / (1.0 + np.exp(-t1))
    emb = (t1 @ np.asarray(inputs["t_w2"], f32) + np.asarray(inputs["t_b2"], f32)).astype(f32)

    def cvt(w):
        return np.ascontiguousarray(np.asarray(w, f32).astype(bf16))

    def packvec(v, nt):
        return np.ascontiguousarray(np.asarray(v, f32).reshape(nt, 128).T)

    com = dict(
        up_w1=cvt(inputs["up_w1"]), up_w2=cvt(inputs["up_w2"]),
        dn_w1=cvt(inputs["down_w1"]), dn_w2=cvt(inputs["down_w2"]),
        Wq=cvt(inputs["Wq"][:n_layers]), Wk=cvt(inputs["Wk"][:n_layers]),
        Wv=cvt(inputs["Wv"][:n_layers]), Wo=cvt(inputs["Wo"][:n_layers]),
        Wi=cvt(inputs["Wi"][:n_layers]), Wo2=cvt(inputs["Wo2"][:n_layers]),
        dn_b2=np.ascontiguousarray(np.asarray(inputs["down_b2"], f32).reshape(1, C).T),
    )
    vecs = np.stack([
        np.concatenate([packvec(np.asarray(inputs[k], f32)[l], HT) for k in
                        ("bq", "bk", "bv", "bo", "g1", "b1", "bo2", "g2", "b2", "b2")],
                       axis=1)
        for l in range(n_layers)])
    com["vecs"] = np.ascontiguousarray(vecs.astype(f32))
    com["bi_all"] = np.ascontiguousarray(
        np.stack([packvec(np.asarray(inputs["bi"], f32)[l], FT)
                  for l in range(n_layers)]).astype(f32))
    com["gvec"] = np.ascontiguousarray(np.concatenate([
        packvec(inputs["up_b1"], HT), packvec(inputs["up_b2"], HT),
        packvec(inputs["ln_g"], HT), packvec(inputs["ln_b"], HT),
        packvec(inputs["down_b1"], HT), packvec(inputs["down_b1"], HT)],
        axis=1).astype(f32))

    pos = np.asarray(inputs["pos_emb"], f32)
    mk = {}
    for sh in range(2):
        base = sh * T
        m = np.empty((2, 8, 128, 512), f32)
        for qt in range(2):
            for j in range(8):
                gk = base - W1 + (4 * qt + j) * 128 + np.arange(128)[:, None]
                gq = base + qt * 512 + np.arange(512)[None, :]
                valid = (np.abs(gk - gq) <= W1) & (gk >= 0) & (gk < S)
                m[qt, j] = np.where(valid, 0.0, NEG)
        mk[sh] = np.ascontiguousarray(m.reshape(16, 128, 512).astype(bf16))

    in_maps = []
    for c in range(8):
        b, sh = c // 2, c % 2
        sl = slice(sh * T, (sh + 1) * T)
        im = dict(com)
        im["xT"] = np.ascontiguousarray(x[b, sl].T.astype(bf16))
        im["pe"] = np.ascontiguousarray((pos[sl] + emb[b][None, :]).T.astype(f32))
        im["mks"] = mk[sh]
        in_maps.append(im)
    return in_maps


def kernel(**inputs):
    from concourse.bass_utils import run_bass_kernel_spmd

    n_layers = L
    if n_layers not in _CACHED:
        _CACHED[n_layers] = _build(n_layers)
    nc = _CACHED[n_layers]
    in_maps = _host_prep(inputs, n_layers)
    trace = os.environ.get("KERNEL_TRACE", "0") == "1"
    tmpdir = os.environ.get("KERNEL_TMPDIR") or None
    res = run_bass_kernel_spmd(nc, in_maps, list(range(8)), trace=trace,
                               tmpdir=tmpdir)
    if getattr(res, "exec_time_ns", None):
        print(f"HW exec time: {res.exec_time_ns} ns")
    out = np.empty((B, S, C), np.float32)
    for c in range(8):
        b, sh = c // 2, c % 2
        out[b, sh * T:(sh + 1) * T, :] = res.results[c]["y"].T
    return out


# revision 16
# speedup vs baseline: 1.2046x; 1.0214x over previous
"""Longformer encoder (12-layer, sliding-window attention) on 8 Trainium2 cores.

Sharding: (batch=4) x (seq half=2) -> 8 cores; 1024 tokens/core.
Sliding-window attention (+-256) uses a per-layer K/V halo exchange between
the two cores of each batch pair via a 2-rank AllGather.

On-device layout is feature-major: activations [feature_partition, token].
  - projections:  psum[outf, tok] = W[inf, outf].T @ h[inf, tok]   (W stationary)
  - V:            psum[tok, outf] = h[inf, tok].T @ Wv[inf, outf]  (h stationary)
  - scores:       psum[ktok, qtok] = K[hd, ktok].T @ Q[hd, qtok]
  - PV:           psum[hd(+1), qtok] = Vtok[ktok, hd+1].T @ P[ktok, qtok]
                  (extra all-ones column of Vtok yields the softmax denominator)
All matmuls bf16 with fp32 PSUM accumulation; layernorm/softmax math fp32.
"""

import os
import numpy as np
import ml_dtypes

B, S, C = 4, 2048, 128
H, NH, HD, FF, W1 = 768, 12, 64, 3072, 256
L = int(os.environ.get("KERNEL_NL", "12"))
T = 1024            # tokens per core
KP = T + 2 * W1     # padded key range per core (1536)
HT = H // 128       # feature tiles (6)
FT = FF // 128      # ffn feature tiles (24)
VH = HD + 1         # v columns per head incl ones column (65)
VW = NH * VH        # v row width per token tile (780)
EPS = 1e-5
NEG = -30000.0
ISQ = float(1.0 / np.sqrt(HD))

bf16 = ml_dtypes.bfloat16

_CACHED = {}


def _build(n_layers):
    import concourse.bacc as bacc
    import concourse.mybir as mybir
    from concourse import tile
    from contextlib import ExitStack

    dt = mybir.dt
    AF = mybir.ActivationFunctionType
    OP = mybir.AluOpType

    nc = bacc.Bacc(None, target_bir_lowering=False, debug=False)

    # ---------------- DRAM I/O ----------------
    xT = nc.dram_tensor("xT", [C, T], dt.bfloat16, kind="ExternalInput")
    pe = nc.dram_tensor("pe", [H, T], dt.float32, kind="ExternalInput")
    mks = nc.dram_tensor("mks", [16, 128, 512], dt.bfloat16, kind="ExternalInput")

    up_w1 = nc.dram_tensor("up_w1", [C, H], dt.bfloat16, kind="ExternalInput")
    up_w2 = nc.dram_tensor("up_w2", [H, H], dt.bfloat16, kind="ExternalInput")
    dn_w1 = nc.dram_tensor("dn_w1", [H, H], dt.bfloat16, kind="ExternalInput")
    dn_w2 = nc.dram_tensor("dn_w2", [H, C], dt.bfloat16, kind="ExternalInput")

    Wq = nc.dram_tensor("Wq", [n_layers, H, H], dt.bfloat16, kind="ExternalInput")
    Wk = nc.dram_tensor("Wk", [n_layers, H, H], dt.bfloat16, kind="ExternalInput")
    Wv = nc.dram_tensor("Wv", [n_layers, H, H], dt.bfloat16, kind="ExternalInput")
    Wo = nc.dram_tensor("Wo", [n_layers, H, H], dt.bfloat16, kind="ExternalInput")
    Wi = nc.dram_tensor("Wi", [n_layers, H, FF], dt.bfloat16, kind="ExternalInput")
    Wo2 = nc.dram_tensor("Wo2", [n_layers, FF, H], dt.bfloat16, kind="ExternalInput")

    # per-feature vectors, host-packed as [128, ntiles] (column j = feats 128j:128j+128)
    # order: bq bk bv bo g1 b1 bo2 g2 b2 pad
    vecs = nc.dram_tensor("vecs", [n_layers, 128, 10 * HT], dt.float32, kind="ExternalInput")
    bi_all = nc.dram_tensor("bi_all", [n_layers, 128, FT], dt.float32, kind="ExternalInput")
    # order: up_b1 up_b2 ln_g ln_b dn_b1 pad
    gvec = nc.dram_tensor("gvec", [128, 6 * HT], dt.float32, kind="ExternalInput")
    dn_b2 = nc.dram_tensor("dn_b2", [128, 1], dt.float32, kind="ExternalInput")

    y = nc.dram_tensor("y", [C, T], dt.float32, kind="ExternalOutput")

    # halo exchange buffers: h-stream edges (halo K/V projected locally)
    KSEG = HT * 128 * W1
    SEND_N = 2 * KSEG
    cc_send = nc.dram_tensor("cc_send", [SEND_N], dt.bfloat16)
    cc_recv = nc.dram_tensor("cc_recv", [2, SEND_N], dt.bfloat16)
    K_OFF = [0, KSEG]

    with tile.TileContext(nc) as tc, ExitStack() as ctx:
        pp = ctx.enter_context(tc.tile_pool(name="persist", bufs=1))
        wp = ctx.enter_context(tc.tile_pool(name="w768", bufs=6))
        bp = ctx.enter_context(tc.tile_pool(name="bias", bufs=2))
        fp = ctx.enter_context(tc.tile_pool(name="ffmid", bufs=2))
        sp = ctx.enter_context(tc.tile_pool(name="scratch", bufs=1))
        rp = ctx.enter_context(tc.tile_pool(name="rows", bufs=2))
        pbp = ctx.enter_context(tc.tile_pool(name="pbuf", bufs=3))
        psA = ctx.enter_context(tc.tile_pool(name="psA", bufs=2, space="PSUM"))

        # ---------------- persistent tiles ----------------
        h = pp.tile([128, HT * T], dt.float32, tag="h")          # residual stream
        hb = pp.tile([128, HT * T], dt.bfloat16, tag="hb")       # bf16 copy of stream
        qb = pp.tile([128, HT * T], dt.bfloat16, tag="qb")       # Q (feature-major)
        ob = pp.tile([128, HT * T], dt.bfloat16, tag="ob")       # attn out (feature-major)
        # K padded, feature-major, split [left pad | interior | right pad]
        Kpl = pp.tile([128, HT * W1], dt.bfloat16, tag="Kpl")
        Kpm = pp.tile([128, HT * T], dt.bfloat16, tag="Kpm")
        Kpr = pp.tile([128, HT * W1], dt.bfloat16, tag="Kpr")
        # V padded, token-major (65-wide head slots), split by token tiles 0-1|2-9|10-11
        Vpl = pp.tile([128, 2 * VW], dt.bfloat16, tag="Vpl")
        Vpm = pp.tile([128, 8 * VW], dt.bfloat16, tag="Vpm")
        Vpr = pp.tile([128, 2 * VW], dt.bfloat16, tag="Vpr")
        hbh = pp.tile([128, HT * 512], dt.bfloat16, tag="hbh")   # halo h [left|right]
        msk = pp.tile([128, 16 * 512], dt.bfloat16, tag="msk")   # additive masks
        ones_c = pp.tile([128, 1], dt.bfloat16, tag="ones_c")    # [128,1] ones (stats lhsT)
        ones_r = pp.tile([128, 128], dt.bfloat16, tag="ones_r")  # ones (bcast lhsT slices)
        gv = pp.tile([128, 6 * HT], dt.float32, tag="gv")
        dnb2 = pp.tile([128, 1], dt.float32, tag="dnb2")

        def hs(ft, qt=None):
            if qt is None:
                return slice(ft * T, (ft + 1) * T)
            return slice(ft * T + qt * 512, ft * T + qt * 512 + 512)

        def k_ap(ft, kt):  # lhsT AP [128, 128] for padded key tile kt (0..11)
            if kt < 2:
                return Kpl[:, ft * W1 + kt * 128: ft * W1 + (kt + 1) * 128]
            if kt < 10:
                return Kpm[:, ft * T + (kt - 2) * 128: ft * T + (kt - 1) * 128]
            return Kpr[:, ft * W1 + (kt - 10) * 128: ft * W1 + (kt - 9) * 128]

        def v_ap(i):  # full [128, VW] token tile i of padded V (0..11)
            if i < 2:
                return Vpl[:, i * VW:(i + 1) * VW]
            if i < 10:
                return Vpm[:, (i - 2) * VW:(i - 1) * VW]
            return Vpr[:, (i - 10) * VW:(i - 9) * VW]

        nc.vector.memset(ones_c[:], 1.0)
        nc.vector.memset(ones_r[:], 1.0)
        for i in range(12):
            vre = v_ap(i).rearrange("p (n c) -> p n c", c=VH)
            nc.vector.memset(vre[:, :, HD:HD + 1], 1.0)

        nc.sync.dma_start(out=gv[:], in_=gvec[:])
        nc.sync.dma_start(out=dnb2[:], in_=dn_b2[:])
        for j in range(16):
            nc.sync.dma_start(out=msk[:, j * 512:(j + 1) * 512], in_=mks[j])

        # ---------------- helpers ----------------
        def w_proj(w_dram, rhs_cols, out_fn, n_k=HT, nq=2):
            """psum[mt][qt] = W.T @ rhs ; out_fn(mt, qt, psum) evicts."""
            wt = []
            for k in range(n_k):
                t = wp.tile([128, H], dt.bfloat16, tag="w768")
                nc.sync.dma_start(out=t[:], in_=w_dram[k * 128:(k + 1) * 128, :])
                wt.append(t)
            for mt in range(HT):
                for qt in range(nq):
                    pt = psA.tile([128, 512], dt.float32, tag="work")
                    for k in range(n_k):
                        nc.tensor.matmul(
                            pt[:], wt[k][:, mt * 128:(mt + 1) * 128],
                            rhs_cols(k, qt),
                            start=(k == 0), stop=(k == n_k - 1))
                    out_fn(mt, qt, pt)

        # layernorm: reads/writes h in place, writes hb
        def layernorm(g_col, b_col, lp):
            ub = sp.tile([128, HT * T], dt.bfloat16, tag="scr")
            for ft in range(HT):
                nc.scalar.activation(ub[:, hs(ft)], h[:, hs(ft)], AF.Copy)
            for qt in range(2):
                mp = lp.tile([1, 512], dt.float32, tag="statm")
                for ft in range(HT):
                    nc.tensor.matmul(mp[:], ones_c[:], ub[:, hs(ft, qt)],
                                     start=(ft == 0), stop=(ft == HT - 1))
                mrow = rp.tile([1, 512], dt.bfloat16, tag="mrow")
                nc.scalar.activation(mrow[:], mp[:], AF.Copy, scale=1.0 / H)
                m2 = rp.tile([1, 512], dt.float32, tag="rowA")
                nc.scalar.activation(m2[:], mp[:], AF.Square, scale=1.0 / H)
                for ft in range(HT):
                    nc.scalar.square(ub[:, hs(ft, qt)], ub[:, hs(ft, qt)])
                sq = lp.tile([1, 512], dt.float32, tag="stats")
                for ft in range(HT):
                    nc.tensor.matmul(sq[:], ones_c[:], ub[:, hs(ft, qt)],
                                     start=(ft == 0), stop=(ft == HT - 1))
                var = rp.tile([1, 512], dt.float32, tag="rowB")
                nc.vector.scalar_tensor_tensor(var[:], sq[:], 1.0 / H, m2[:],
                                               OP.mult, OP.subtract)
                nc.vector.tensor_scalar_add(var[:], var[:], EPS)
                rec = rp.tile([1, 512], dt.float32, tag="rowA", name="rec")
                nc.scalar.activation(rec[:], var[:], AF.Ln)
                rrow = rp.tile([1, 512], dt.bfloat16, tag="mrow", name="rrow")
                nc.scalar.activation(rrow[:], rec[:], AF.Exp, scale=-0.5)
                mb = lp.tile([128, 512], dt.float32, tag="mb")
                nc.tensor.matmul(mb[:], ones_r[0:1, :], mrow[:], start=True, stop=True)
                rb = lp.tile([128, 512], dt.float32, tag="rb")
                nc.tensor.matmul(rb[:], ones_r[0:1, :], rrow[:], start=True, stop=True)
                for ft in range(HT):
                    sl = hs(ft, qt)
                    nc.vector.tensor_sub(h[:, sl], h[:, sl], mb[:])
                    nc.vector.scalar_tensor_tensor(h[:, sl], h[:, sl],
                                                   g_col(ft), rb[:], OP.mult, OP.mult)
                    nc.vector.tensor_scalar_add(h[:, sl], h[:, sl], b_col(ft))
                    nc.scalar.activation(hb[:, sl], h[:, sl], AF.Copy)

        # ---------------- input projection ----------------
        # h <- pos_emb + t_emb  (host-combined), then += up-proj
        for ft in range(HT):
            nc.sync.dma_start(out=h[:, hs(ft)], in_=pe[ft * 128:(ft + 1) * 128, :])
        xb = sp.tile([128, T], dt.bfloat16, tag="xb")
        nc.sync.dma_start(out=xb[:], in_=xT[:])

        w1t = wp.tile([128, H], dt.bfloat16, tag="w768")
        nc.sync.dma_start(out=w1t[:], in_=up_w1[:])
        t1 = sp.tile([128, HT * T], dt.bfloat16, tag="scr")
        for mt in range(HT):
            for qt in range(2):
                pt = psA.tile([128, 512], dt.float32, tag="work")
                nc.tensor.matmul(pt[:], w1t[:, mt * 128:(mt + 1) * 128],
                                 xb[:, qt * 512:qt * 512 + 512], start=True, stop=True)
                nc.scalar.activation(t1[:, hs(mt, qt)], pt[:], AF.Tanh,
                                     bias=gv[:, 0 * HT + mt:0 * HT + mt + 1])

        def up2_out(mt, qt, pt):
            sl = hs(mt, qt)
            nc.vector.scalar_tensor_tensor(
                h[:, sl], pt[:], gv[:, 1 * HT + mt:1 * HT + mt + 1], h[:, sl],
                OP.add, OP.add)
        w_proj(up_w2, lambda k, qt: t1[:, hs(k, qt)], up2_out)

        with tc.tile_pool(name="lnps", bufs=1, space="PSUM") as lp0:
            layernorm(lambda ft: gv[:, 2 * HT + ft:2 * HT + ft + 1],
                      lambda ft: gv[:, 3 * HT + ft:3 * HT + ft + 1], lp0)

        # ---------------- encoder layers ----------------
        for l in range(n_layers):
            bv_t = bp.tile([128, 10 * HT], dt.float32, tag="bvec")
            nc.sync.dma_start(out=bv_t[:], in_=vecs[l])
            bi_t = bp.tile([128, FT], dt.float32, tag="bivec")
            nc.sync.dma_start(out=bi_t[:], in_=bi_all[l])

            def vcol(i, ft):
                return bv_t[:, i * HT + ft:i * HT + ft + 1]

            # --- h-edge halo exchange (overlaps with Q/K/V projections) ---
            for ft in range(HT):
                nc.sync.dma_start(
                    out=cc_send[K_OFF[0] + ft * 128 * W1:K_OFF[0] + (ft + 1) * 128 * W1]
                    .rearrange("(p t) -> p t", p=128),
                    in_=hb[:, ft * T:ft * T + W1])
                nc.sync.dma_start(
                    out=cc_send[K_OFF[1] + ft * 128 * W1:K_OFF[1] + (ft + 1) * 128 * W1]
                    .rearrange("(p t) -> p t", p=128),
                    in_=hb[:, ft * T + T - W1:(ft + 1) * T])
            nc.gpsimd.collective_compute(
                "AllGather", OP.bypass, ins=[cc_send[:]], outs=[cc_recv[:]],
                replica_groups=[[0, 1], [2, 3], [4, 5], [6, 7]])
            # my left halo <- rank0's right edge ; my right halo <- rank1's left edge
            for ft in range(HT):
                nc.sync.dma_start(
                    out=hbh[:, ft * 512:ft * 512 + W1],
                    in_=cc_recv[0, K_OFF[1] + ft * 128 * W1:K_OFF[1] + (ft + 1) * 128 * W1]
                    .rearrange("(p t) -> p t", p=128))
                nc.sync.dma_start(
                    out=hbh[:, ft * 512 + W1:(ft + 1) * 512],
                    in_=cc_recv[1, K_OFF[0] + ft * 128 * W1:K_OFF[0] + (ft + 1) * 128 * W1]
                    .rearrange("(p t) -> p t", p=128))

            # --- Q, K projections (feature-major) ---
            def q_out(mt, qt, pt):
                nc.scalar.activation(qb[:, hs(mt, qt)], pt[:], AF.Identity,
                                     bias=vcol(0, mt))
            w_proj(Wq[l], lambda k, qt: hb[:, hs(k, qt)], q_out)

            def k_out(mt, qt, pt):
                if qt < 2:
                    sl = slice(mt * T + qt * 512, mt * T + qt * 512 + 512)
                    nc.scalar.activation(Kpm[:, sl], pt[:], AF.Identity, bias=vcol(1, mt))
                else:
                    nc.scalar.activation(Kpl[:, mt * W1:(mt + 1) * W1], pt[:, 0:W1],
                                         AF.Identity, bias=vcol(1, mt))
                    nc.scalar.activation(Kpr[:, mt * W1:(mt + 1) * W1], pt[:, W1:512],
                                         AF.Identity, bias=vcol(1, mt))

            def k_rhs(k, qt):
                if qt < 2:
                    return hb[:, hs(k, qt)]
                return hbh[:, k * 512:(k + 1) * 512]
            w_proj(Wk[l], k_rhs, k_out, nq=3)

            # --- V projection (token-major: h stationary) ---
            wvt = []
            for k in range(HT):
                t = wp.tile([128, H], dt.bfloat16, tag="w768")
                nc.sync.dma_start(out=t[:], in_=Wv[l, k * 128:(k + 1) * 128, :])
                wvt.append(t)
            def v_stat(vt, k):  # stationary h slice for padded v token tile vt
                if vt < 2:
                    return hbh[:, k * 512 + vt * 128:k * 512 + (vt + 1) * 128]
                if vt < 10:
                    tt = vt - 2
                    return hb[:, k * T + tt * 128:k * T + tt * 128 + 128]
                return hbh[:, k * 512 + W1 + (vt - 10) * 128:k * 512 + W1 + (vt - 9) * 128]

            for vt in range(12):
                for n0, nn in ((0, 512), (512, 256)):
                    pt = psA.tile([128, 512], dt.float32, tag="work")
                    for k in range(HT):
                        nc.tensor.matmul(
                            pt[:, :nn], v_stat(vt, k), wvt[k][:, n0:n0 + nn],
                            start=(k == 0), stop=(k == HT - 1))
                    dst = v_ap(vt).rearrange("p (n c) -> p n c", c=VH)
                    h0, nh_ = n0 // HD, nn // HD
                    src = pt[:, :nn].rearrange("p (n c) -> p n c", c=HD)
                    nc.scalar.activation(dst[:, h0:h0 + nh_, 0:HD], src[:], AF.Copy)

            # --- attention per head ---
            att_cm = tc.tile_pool(name="attps", bufs=2, space="PSUM")
            att_ps = att_cm.__enter__()
            for hd_i in range(NH):
                ft, r0 = hd_i // 2, (hd_i % 2) * HD
                for qt in range(2):
                    pb = pbp.tile([128, 8 * 512], dt.bfloat16, tag="pb")
                    for j in range(8):
                        kt = 4 * qt + j
                        sc = psA.tile([128, 512], dt.float32, tag="work")
                        nc.tensor.matmul(
                            sc[:], k_ap(ft, kt)[r0:r0 + HD, :],
                            qb[r0:r0 + HD, hs(ft, qt)],
                            start=True, stop=True)
                        pj = pb[:, j * 512:(j + 1) * 512]
                        nc.vector.tensor_add(
                            pj, sc[:], msk[:, (qt * 8 + j) * 512:(qt * 8 + j + 1) * 512])
                    nc.scalar.activation(pb[:], pb[:], AF.Exp, scale=ISQ)
                    pv = att_ps.tile([VH, 512], dt.float32, tag="pv")
                    for j in range(8):
                        nc.tensor.matmul(
                            pv[:], v_ap(4 * qt + j)[:, hd_i * VH:(hd_i + 1) * VH],
                            pb[:, j * 512:(j + 1) * 512],
                            start=(j == 0), stop=(j == 7))
                    r128 = rp.tile([128, 1024], dt.float32, tag="r128")
                    rb128 = rp.tile([128, 512], dt.bfloat16, tag="rb128")
                    nc.scalar.activation(r128[0:1, 0:512], pv[HD:VH, :], AF.Copy)
                    nc.vector.reciprocal_approx_fast(r128[0:1, 512:1024],
                                                     r128[0:1, 0:512])
                    nc.vector.tensor_copy(rb128[0:1, :], r128[0:1, 512:1024])
                    bc = att_ps.tile([HD, 512], dt.float32, tag="bcr")
                    nc.tensor.matmul(bc[:], ones_r[0:1, 0:HD], rb128[0:1, :],
                                     start=True, stop=True)
                    bcs = rp.tile([HD, 512], dt.bfloat16, tag="otmp", name="bcs")
                    nc.scalar.activation(bcs[:], bc[:], AF.Copy)
                    ot = rp.tile([HD, 512], dt.bfloat16, tag="otmp")
                    nc.vector.tensor_mul(ot[:], pv[0:HD, :], bcs[:])
                    nc.scalar.activation(
                        ob[r0:r0 + HD, hs(ft, qt)], ot[:], AF.Identity,
                        bias=vcol(2, ft)[r0:r0 + HD, :])

            att_cm.__exit__(None, None, None)

            # --- O projection + residual + LN1 ---
            def o_out(mt, qt, pt):
                sl = hs(mt, qt)
                nc.vector.scalar_tensor_tensor(h[:, sl], pt[:], vcol(3, mt), h[:, sl],
                                               OP.add, OP.add)
            w_proj(Wo[l], lambda k, qt: ob[:, hs(k, qt)], o_out)
            with tc.tile_pool(name="lnps", bufs=1, space="PSUM") as lp1:
                layernorm(lambda ft: vcol(4, ft), lambda ft: vcol(5, ft), lp1)

            # --- FFN ---
            ffn_cm = tc.tile_pool(name="ffps", bufs=1, space="PSUM")
            ffn_ps = ffn_cm.__enter__()
            for qt in range(2):
                fpt = [ffn_ps.tile([128, 512], dt.float32, tag=f"ff2_{m}", name=f"ff2_{m}") for m in range(HT)]
                for ch in range(6):  # 6 chunks of 4 mid tiles (512 cols of FF)
                    wi_ch = []
                    for k in range(HT):
                        t = wp.tile([128, 512], dt.bfloat16, tag="wi")
                        nc.sync.dma_start(
                            out=t[:],
                            in_=Wi[l, k * 128:(k + 1) * 128, ch * 512:(ch + 1) * 512])
                        wi_ch.append(t)
                    fmid = fp.tile([128, 4 * 512], dt.bfloat16, tag="fmid")
                    for mi in range(4):
                        mt = ch * 4 + mi
                        pt = psA.tile([128, 512], dt.float32, tag="work")
                        for k in range(HT):
                            nc.tensor.matmul(
                                pt[:], wi_ch[k][:, mi * 128:(mi + 1) * 128],
                                hb[:, hs(k, qt)],
                                start=(k == 0), stop=(k == HT - 1))
                        nc.scalar.activation(fmid[:, mi * 512:(mi + 1) * 512], pt[:],
                                             AF.Gelu, bias=bi_t[:, mt:mt + 1])
                    wo2_ch = []
                    for mi in range(4):
                        t = wp.tile([128, H], dt.bfloat16, tag="w768")
                        nc.sync.dma_start(
                            out=t[:],
                            in_=Wo2[l, (ch * 4 + mi) * 128:(ch * 4 + mi + 1) * 128, :])
                        wo2_ch.append(t)
                    for m in range(HT):
                        for mi in range(4):
                            kt = ch * 4 + mi
                            nc.tensor.matmul(
                                fpt[m][:], wo2_ch[mi][:, m * 128:(m + 1) * 128],
                                fmid[:, mi * 512:(mi + 1) * 512],
                                start=(kt == 0), stop=(kt == FT - 1))
                for m in range(HT):
                    sl = hs(m, qt)
                    nc.vector.scalar_tensor_tensor(h[:, sl], fpt[m][:], vcol(6, m),
                                                   h[:, sl], OP.add, OP.add)
            ffn_cm.__exit__(None, None, None)
            with tc.tile_pool(name="lnps", bufs=1, space="PSUM") as lp2:
                layernorm(lambda ft: vcol(7, ft), lambda ft: vcol(8, ft), lp2)

        # ---------------- output projection ----------------
        t2 = sp.tile([128, HT * T], dt.bfloat16, tag="scr")

        def d1_out(mt, qt, pt):
            nc.scalar.activation(t2[:, hs(mt, qt)], pt[:], AF.Tanh,
                                 bias=gv[:, 4 * HT + mt:4 * HT + mt + 1])
        w_proj(dn_w1, lambda k, qt: hb[:, hs(k, qt)], d1_out)

        w2t = wp.tile([128, HT * C], dt.bfloat16, tag="w768")
        for k in range(HT):
            nc.sync.dma_start(out=w2t[:, k * C:(k + 1) * C],
                              in_=dn_w2[k * 128:(k + 1) * 128, :])
        for qt in range(2):
            pt = psA.tile([128, 512], dt.float32, tag="work")
            for k in range(HT):
                nc.tensor.matmul(pt[:], w2t[:, k * C:(k + 1) * C],
                                 t2[:, hs(k, qt)], start=(k == 0), stop=(k == HT - 1))
            yo = rp.tile([128, 512], dt.float32, tag="r128", name="yout")
            nc.scalar.activation(yo[:], pt[:], AF.Identity, bias=dnb2[:])
            nc.sync.dma_start(out=y[:, qt * 512:qt * 512 + 512], in_=yo[:])

    nc.compile()
    return nc


def _host_prep(inputs, n_layers):
    f32 = np.float32
    x = np.asarray(inputs["x"], f32)
    ts = np.asarray(inputs["timesteps"])
    half = C // 2
    freqs = np.exp(-np.log(10000.0) * np.arange(half, dtype=f32) / half)
    a = ts.astype(f32)[:, None] * freqs[None, :]
    emb0 = np.concatenate([np.cos(a), np.sin(a)], -1).astype(f32)
    t1 = emb0 @ np.asarray(inputs["t_w1"], f32) + np.asarray(inputs["t_b1"], f32)
    t1 = t1 / (1.0 + np.exp(-t1))
    emb = (t1 @ np.asarray(inputs["t_w2"], f32) + np.asarray(inputs["t_b2"], f32)).astype(f32)

    def cvt(w):
        return np.ascontiguousarray(np.asarray(w, f32).astype(bf16))

    def packvec(v, nt):
        return np.ascontiguousarray(np.asarray(v, f32).reshape(nt, 128).T)

    com = dict(
        up_w1=cvt(inputs["up_w1"]), up_w2=cvt(inputs["up_w2"]),
        dn_w1=cvt(inputs["down_w1"]), dn_w2=cvt(inputs["down_w2"]),
        Wq=cvt(inputs["Wq"][:n_layers]), Wk=cvt(inputs["Wk"][:n_layers]),
        Wv=cvt(inputs["Wv"][:n_layers]), Wo=cvt(inputs["Wo"][:n_layers]),
        Wi=cvt(inputs["Wi"][:n_layers]), Wo2=cvt(inputs["Wo2"][:n_layers]),
        dn_b2=np.ascontiguousarray(np.asarray(inputs["down_b2"], f32).reshape(1, C).T),
    )
    vecs = np.stack([
        np.concatenate([packvec(np.asarray(inputs[k], f32)[l], HT) for k in
                        ("bq", "bk", "bv", "bo", "g1", "b1", "bo2", "g2", "b2", "b2")],
                       axis=1)
        for l in range(n_layers)])
    com["vecs"] = np.ascontiguousarray(vecs.astype(f32))
    com["bi_all"] = np.ascontiguousarray(
        np.stack([packvec(np.asarray(inputs["bi"], f32)[l], FT)
                  for l in range(n_layers)]).astype(f32))
    com["gvec"] = np.ascontiguousarray(np.concatenate([
        packvec(inputs["up_b1"], HT), packvec(inputs["up_b2"], HT),
        packvec(inputs["ln_g"], HT), packvec(inputs["ln_b"], HT),
        packvec(inputs["down_b1"], HT), packvec(inputs["down_b1"], HT)],
        axis=1).astype(f32))

    pos = np.asarray(inputs["pos_emb"], f32)
    mk = {}
    for sh in range(2):
        base = sh * T
        m = np.empty((2, 8, 128, 512), f32)
        for qt in range(2):
            for j in range(8):
                gk = base - W1 + (4 * qt + j) * 128 + np.arange(128)[:, None]
                gq = base + qt * 512 + np.arange(512)[None, :]
                valid = (np.abs(gk - gq) <= W1) & (gk >= 0) & (gk < S)
                m[qt, j] = np.where(valid, 0.0, NEG)
        mk[sh] = np.ascontiguousarray(m.reshape(16, 128, 512).astype(bf16))

    in_maps = []
    for c in range(8):
        b, sh = c // 2, c % 2
        sl = slice(sh * T, (sh + 1) * T)
        im = dict(com)
        im["xT"] = np.ascontiguousarray(x[b, sl].T.astype(bf16))
        im["pe"] = np.ascontiguousarray((pos[sl] + emb[b][None, :]).T.astype(f32))
        im["mks"] = mk[sh]
        in_maps.append(im)
    return in_maps


def kernel(**inputs):
    from concourse.bass_utils import run_bass_kernel_spmd

    n_layers = L
    if n_layers not in _CACHED:
        _CACHED[n_layers] = _build(n_layers)
    nc = _CACHED[n_layers]
    in_maps = _host_prep(inputs, n_layers)
    trace = os.environ.get("KERNEL_TRACE", "0") == "1"
    tmpdir = os.environ.get("KERNEL_TMPDIR") or None
    res = run_bass_kernel_spmd(nc, in_maps, list(range(8)), trace=trace,
                               tmpdir=tmpdir)
    if getattr(res, "exec_time_ns", None):
        print(f"HW exec time: {res.exec_time_ns} ns")
    out = np.empty((B, S, C), np.float32)
    for c in range(8):
        b, sh = c // 2, c % 2
        out[b, sh * T:(sh + 1) * T, :] = res.results[c]["y"].T
    return out


# revision 17
# speedup vs baseline: 1.2131x; 1.0071x over previous
"""Longformer encoder (12-layer, sliding-window attention) on 8 Trainium2 cores.

Sharding: (batch=4) x (seq half=2) -> 8 cores; 1024 tokens/core.
Sliding-window attention (+-256) uses a per-layer K/V halo exchange between
the two cores of each batch pair via a 2-rank AllGather.

On-device layout is feature-major: activations [feature_partition, token].
  - projections:  psum[outf, tok] = W[inf, outf].T @ h[inf, tok]   (W stationary)
  - V:            psum[tok, outf] = h[inf, tok].T @ Wv[inf, outf]  (h stationary)
  - scores:       psum[ktok, qtok] = K[hd, ktok].T @ Q[hd, qtok]
  - PV:           psum[hd(+1), qtok] = Vtok[ktok, hd+1].T @ P[ktok, qtok]
                  (extra all-ones column of Vtok yields the softmax denominator)
All matmuls bf16 with fp32 PSUM accumulation; layernorm/softmax math fp32.
"""

import os
import numpy as np
import ml_dtypes

B, S, C = 4, 2048, 128
H, NH, HD, FF, W1 = 768, 12, 64, 3072, 256
L = int(os.environ.get("KERNEL_NL", "12"))
T = 1024            # tokens per core
KP = T + 2 * W1     # padded key range per core (1536)
HT = H // 128       # feature tiles (6)
FT = FF // 128      # ffn feature tiles (24)
VH = HD + 1         # v columns per head incl ones column (65)
VW = NH * VH        # v row width per token tile (780)
EPS = 1e-5
NEG = -30000.0
ISQ = float(1.0 / np.sqrt(HD))

bf16 = ml_dtypes.bfloat16

_CACHED = {}


def _build(n_layers):
    import concourse.bacc as bacc
    import concourse.mybir as mybir
    from concourse import tile
    from contextlib import ExitStack

    dt = mybir.dt
    AF = mybir.ActivationFunctionType
    OP = mybir.AluOpType

    nc = bacc.Bacc(None, target_bir_lowering=False, debug=False)

    # ---------------- DRAM I/O ----------------
    xT = nc.dram_tensor("xT", [C, T], dt.bfloat16, kind="ExternalInput")
    pe = nc.dram_tensor("pe", [H, T], dt.float32, kind="ExternalInput")
    mks = nc.dram_tensor("mks", [16, 128, 512], dt.bfloat16, kind="ExternalInput")

    up_w1 = nc.dram_tensor("up_w1", [C, H], dt.bfloat16, kind="ExternalInput")
    up_w2 = nc.dram_tensor("up_w2", [H, H], dt.bfloat16, kind="ExternalInput")
    dn_w1 = nc.dram_tensor("dn_w1", [H, H], dt.bfloat16, kind="ExternalInput")
    dn_w2 = nc.dram_tensor("dn_w2", [H, C], dt.bfloat16, kind="ExternalInput")

    Wq = nc.dram_tensor("Wq", [n_layers, H, H], dt.bfloat16, kind="ExternalInput")
    Wk = nc.dram_tensor("Wk", [n_layers, H, H], dt.bfloat16, kind="ExternalInput")
    Wv = nc.dram_tensor("Wv", [n_layers, H, H], dt.bfloat16, kind="ExternalInput")
    Wo = nc.dram_tensor("Wo", [n_layers, H, H], dt.bfloat16, kind="ExternalInput")
    Wi = nc.dram_tensor("Wi", [n_layers, H, FF], dt.bfloat16, kind="ExternalInput")
    Wo2 = nc.dram_tensor("Wo2", [n_layers, FF, H], dt.bfloat16, kind="ExternalInput")

    # per-feature vectors, host-packed as [128, ntiles] (column j = feats 128j:128j+128)
    # order: bq bk bv bo g1 b1 bo2 g2 b2 pad
    vecs = nc.dram_tensor("vecs", [n_layers, 128, 10 * HT], dt.float32, kind="ExternalInput")
    bi_all = nc.dram_tensor("bi_all", [n_layers, 128, FT], dt.float32, kind="ExternalInput")
    # order: up_b1 up_b2 ln_g ln_b dn_b1 pad
    gvec = nc.dram_tensor("gvec", [128, 6 * HT], dt.float32, kind="ExternalInput")
    dn_b2 = nc.dram_tensor("dn_b2", [128, 1], dt.float32, kind="ExternalInput")

    y = nc.dram_tensor("y", [C, T], dt.float32, kind="ExternalOutput")

    # halo exchange buffers: h-stream edges (halo K/V projected locally)
    KSEG = HT * 128 * W1
    SEND_N = 2 * KSEG
    cc_send = nc.dram_tensor("cc_send", [SEND_N], dt.bfloat16)
    cc_recv = nc.dram_tensor("cc_recv", [2, SEND_N], dt.bfloat16)
    K_OFF = [0, KSEG]

    with tile.TileContext(nc) as tc, ExitStack() as ctx:
        pp = ctx.enter_context(tc.tile_pool(name="persist", bufs=1))
        wp = ctx.enter_context(tc.tile_pool(name="w768", bufs=6))
        bp = ctx.enter_context(tc.tile_pool(name="bias", bufs=2))
        fp = ctx.enter_context(tc.tile_pool(name="ffmid", bufs=2))
        sp = ctx.enter_context(tc.tile_pool(name="scratch", bufs=1))
        rp = ctx.enter_context(tc.tile_pool(name="rows", bufs=2))
        pbp = ctx.enter_context(tc.tile_pool(name="pbuf", bufs=3))
        psA = ctx.enter_context(tc.tile_pool(name="psA", bufs=2, space="PSUM"))

        # ---------------- persistent tiles ----------------
        h = pp.tile([128, HT * T], dt.float32, tag="h")          # residual stream
        hb = pp.tile([128, HT * T], dt.bfloat16, tag="hb")       # bf16 copy of stream
        qb = pp.tile([128, HT * T], dt.bfloat16, tag="qb")       # Q (feature-major)
        ob = pp.tile([128, HT * T], dt.bfloat16, tag="ob")       # attn out (feature-major)
        # K padded, feature-major, split [left pad | interior | right pad]
        Kpl = pp.tile([128, HT * W1], dt.bfloat16, tag="Kpl")
        Kpm = pp.tile([128, HT * T], dt.bfloat16, tag="Kpm")
        Kpr = pp.tile([128, HT * W1], dt.bfloat16, tag="Kpr")
        # V padded, token-major (65-wide head slots), split by token tiles 0-1|2-9|10-11
        Vpl = pp.tile([128, 2 * VW], dt.bfloat16, tag="Vpl")
        Vpm = pp.tile([128, 8 * VW], dt.bfloat16, tag="Vpm")
        Vpr = pp.tile([128, 2 * VW], dt.bfloat16, tag="Vpr")
        hbh = pp.tile([128, HT * 512], dt.bfloat16, tag="hbh")   # halo h [left|right]
        msk = pp.tile([128, 16 * 512], dt.bfloat16, tag="msk")   # additive masks
        ones_c = pp.tile([128, 1], dt.bfloat16, tag="ones_c")    # [128,1] ones (stats lhsT)
        ones_r = pp.tile([128, 128], dt.bfloat16, tag="ones_r")  # ones (bcast lhsT slices)
        gv = pp.tile([128, 6 * HT], dt.float32, tag="gv")
        dnb2 = pp.tile([128, 1], dt.float32, tag="dnb2")

        def hs(ft, qt=None):
            if qt is None:
                return slice(ft * T, (ft + 1) * T)
            return slice(ft * T + qt * 512, ft * T + qt * 512 + 512)

        def k_ap(ft, kt):  # lhsT AP [128, 128] for padded key tile kt (0..11)
            if kt < 2:
                return Kpl[:, ft * W1 + kt * 128: ft * W1 + (kt + 1) * 128]
            if kt < 10:
                return Kpm[:, ft * T + (kt - 2) * 128: ft * T + (kt - 1) * 128]
            return Kpr[:, ft * W1 + (kt - 10) * 128: ft * W1 + (kt - 9) * 128]

        def v_ap(i):  # full [128, VW] token tile i of padded V (0..11)
            if i < 2:
                return Vpl[:, i * VW:(i + 1) * VW]
            if i < 10:
                return Vpm[:, (i - 2) * VW:(i - 1) * VW]
            return Vpr[:, (i - 10) * VW:(i - 9) * VW]

        nc.vector.memset(ones_c[:], 1.0)
        nc.vector.memset(ones_r[:], 1.0)
        for i in range(12):
            vre = v_ap(i).rearrange("p (n c) -> p n c", c=VH)
            nc.vector.memset(vre[:, :, HD:HD + 1], 1.0)

        nc.sync.dma_start(out=gv[:], in_=gvec[:])
        nc.sync.dma_start(out=dnb2[:], in_=dn_b2[:])
        for j in range(16):
            nc.sync.dma_start(out=msk[:, j * 512:(j + 1) * 512], in_=mks[j])

        # ---------------- helpers ----------------
        def w_proj(w_dram, rhs_cols, out_fn, n_k=HT, nq=2):
            """psum[mt][qt] = W.T @ rhs ; out_fn(mt, qt, psum) evicts."""
            wt = []
            for k in range(n_k):
                t = wp.tile([128, H], dt.bfloat16, tag="w768")
                nc.sync.dma_start(out=t[:], in_=w_dram[k * 128:(k + 1) * 128, :])
                wt.append(t)
            for mt in range(HT):
                for qt in range(nq):
                    pt = psA.tile([128, 512], dt.float32, tag="work")
                    for k in range(n_k):
                        nc.tensor.matmul(
                            pt[:], wt[k][:, mt * 128:(mt + 1) * 128],
                            rhs_cols(k, qt),
                            start=(k == 0), stop=(k == n_k - 1))
                    out_fn(mt, qt, pt)

        # layernorm: reads/writes h in place, writes hb
        def layernorm(g_col, b_col, lp):
            ub = sp.tile([128, HT * T], dt.bfloat16, tag="scr")
            for ft in range(HT):
                nc.scalar.activation(ub[:, hs(ft)], h[:, hs(ft)], AF.Copy)
            for qt in range(2):
                mp = lp.tile([1, 512], dt.float32, tag="statm")
                for ft in range(HT):
                    nc.tensor.matmul(mp[:], ones_c[:], ub[:, hs(ft, qt)],
                                     start=(ft == 0), stop=(ft == HT - 1))
                mrow = rp.tile([1, 512], dt.bfloat16, tag="mrow")
                nc.scalar.activation(mrow[:], mp[:], AF.Copy, scale=1.0 / H)
                m2 = rp.tile([1, 512], dt.float32, tag="rowA")
                nc.scalar.activation(m2[:], mp[:], AF.Square, scale=1.0 / H)
                for ft in range(HT):
                    nc.scalar.square(ub[:, hs(ft, qt)], ub[:, hs(ft, qt)])
                sq = lp.tile([1, 512], dt.float32, tag="stats")
                for ft in range(HT):
                    nc.tensor.matmul(sq[:], ones_c[:], ub[:, hs(ft, qt)],
                                     start=(ft == 0), stop=(ft == HT - 1))
                var = rp.tile([1, 512], dt.float32, tag="rowB")
                nc.vector.scalar_tensor_tensor(var[:], sq[:], 1.0 / H, m2[:],
                                               OP.mult, OP.subtract)
                nc.vector.tensor_scalar_add(var[:], var[:], EPS)
                rec = rp.tile([1, 512], dt.float32, tag="rowA", name="rec")
                nc.scalar.activation(rec[:], var[:], AF.Ln)
                rrow = rp.tile([1, 512], dt.bfloat16, tag="mrow", name="rrow")
                nc.scalar.activation(rrow[:], rec[:], AF.Exp, scale=-0.5)
                mb = lp.tile([128, 512], dt.float32, tag="mb")
                nc.tensor.matmul(mb[:], ones_r[0:1, :], mrow[:], start=True, stop=True)
                rb = lp.tile([128, 512], dt.float32, tag="rb")
                nc.tensor.matmul(rb[:], ones_r[0:1, :], rrow[:], start=True, stop=True)
                for ft in range(HT):
                    sl = hs(ft, qt)
                    nc.vector.tensor_sub(h[:, sl], h[:, sl], mb[:])
                    nc.vector.scalar_tensor_tensor(h[:, sl], h[:, sl],
                                                   g_col(ft), rb[:], OP.mult, OP.mult)
                    nc.vector.tensor_scalar_add(h[:, sl], h[:, sl], b_col(ft))
                    nc.scalar.activation(hb[:, sl], h[:, sl], AF.Copy)

        # ---------------- input projection ----------------
        # h <- pos_emb + t_emb  (host-combined), then += up-proj
        for ft in range(HT):
            nc.sync.dma_start(out=h[:, hs(ft)], in_=pe[ft * 128:(ft + 1) * 128, :])
        xb = sp.tile([128, T], dt.bfloat16, tag="xb")
        nc.sync.dma_start(out=xb[:], in_=xT[:])

        w1t = wp.tile([128, H], dt.bfloat16, tag="w768")
        nc.sync.dma_start(out=w1t[:], in_=up_w1[:])
        t1 = sp.tile([128, HT * T], dt.bfloat16, tag="scr")
        for mt in range(HT):
            for qt in range(2):
                pt = psA.tile([128, 512], dt.float32, tag="work")
                nc.tensor.matmul(pt[:], w1t[:, mt * 128:(mt + 1) * 128],
                                 xb[:, qt * 512:qt * 512 + 512], start=True, stop=True)
                nc.scalar.activation(t1[:, hs(mt, qt)], pt[:], AF.Tanh,
                                     bias=gv[:, 0 * HT + mt:0 * HT + mt + 1])

        def up2_out(mt, qt, pt):
            sl = hs(mt, qt)
            nc.vector.scalar_tensor_tensor(
                h[:, sl], pt[:], gv[:, 1 * HT + mt:1 * HT + mt + 1], h[:, sl],
                OP.add, OP.add)
        w_proj(up_w2, lambda k, qt: t1[:, hs(k, qt)], up2_out)

        with tc.tile_pool(name="lnps", bufs=1, space="PSUM") as lp0:
            layernorm(lambda ft: gv[:, 2 * HT + ft:2 * HT + ft + 1],
                      lambda ft: gv[:, 3 * HT + ft:3 * HT + ft + 1], lp0)

        # ---------------- encoder layers ----------------
        for l in range(n_layers):
            bv_t = bp.tile([128, 10 * HT], dt.float32, tag="bvec")
            nc.sync.dma_start(out=bv_t[:], in_=vecs[l])
            bi_t = bp.tile([128, FT], dt.float32, tag="bivec")
            nc.sync.dma_start(out=bi_t[:], in_=bi_all[l])

            def vcol(i, ft):
                return bv_t[:, i * HT + ft:i * HT + ft + 1]

            # --- h-edge halo exchange (overlaps with Q/K/V projections) ---
            for ft in range(HT):
                nc.sync.dma_start(
                    out=cc_send[K_OFF[0] + ft * 128 * W1:K_OFF[0] + (ft + 1) * 128 * W1]
                    .rearrange("(p t) -> p t", p=128),
                    in_=hb[:, ft * T:ft * T + W1])
                nc.sync.dma_start(
                    out=cc_send[K_OFF[1] + ft * 128 * W1:K_OFF[1] + (ft + 1) * 128 * W1]
                    .rearrange("(p t) -> p t", p=128),
                    in_=hb[:, ft * T + T - W1:(ft + 1) * T])
            nc.gpsimd.collective_compute(
                "AllGather", OP.bypass, ins=[cc_send[:]], outs=[cc_recv[:]],
                replica_groups=[[0, 1], [2, 3], [4, 5], [6, 7]])
            # my left halo <- rank0's right edge ; my right halo <- rank1's left edge
            for ft in range(HT):
                nc.sync.dma_start(
                    out=hbh[:, ft * 512:ft * 512 + W1],
                    in_=cc_recv[0, K_OFF[1] + ft * 128 * W1:K_OFF[1] + (ft + 1) * 128 * W1]
                    .rearrange("(p t) -> p t", p=128))
                nc.sync.dma_start(
                    out=hbh[:, ft * 512 + W1:(ft + 1) * 512],
                    in_=cc_recv[1, K_OFF[0] + ft * 128 * W1:K_OFF[0] + (ft + 1) * 128 * W1]
                    .rearrange("(p t) -> p t", p=128))

            # --- Q, K projections (feature-major) ---
            def q_out(mt, qt, pt):
                nc.vector.tensor_scalar_add(qb[:, hs(mt, qt)], pt[:], vcol(0, mt))
            w_proj(Wq[l], lambda k, qt: hb[:, hs(k, qt)], q_out)

            def k_out(mt, qt, pt):
                if qt < 2:
                    sl = slice(mt * T + qt * 512, mt * T + qt * 512 + 512)
                    nc.vector.tensor_scalar_add(Kpm[:, sl], pt[:], vcol(1, mt))
                else:
                    nc.vector.tensor_scalar_add(Kpl[:, mt * W1:(mt + 1) * W1],
                                                pt[:, 0:W1], vcol(1, mt))
                    nc.vector.tensor_scalar_add(Kpr[:, mt * W1:(mt + 1) * W1],
                                                pt[:, W1:512], vcol(1, mt))

            def k_rhs(k, qt):
                if qt < 2:
                    return hb[:, hs(k, qt)]
                return hbh[:, k * 512:(k + 1) * 512]
            w_proj(Wk[l], k_rhs, k_out, nq=3)

            # --- V projection (token-major: h stationary) ---
            wvt = []
            for k in range(HT):
                t = wp.tile([128, H], dt.bfloat16, tag="w768")
                nc.sync.dma_start(out=t[:], in_=Wv[l, k * 128:(k + 1) * 128, :])
                wvt.append(t)
            def v_stat(vt, k):  # stationary h slice for padded v token tile vt
                if vt < 2:
                    return hbh[:, k * 512 + vt * 128:k * 512 + (vt + 1) * 128]
                if vt < 10:
                    tt = vt - 2
                    return hb[:, k * T + tt * 128:k * T + tt * 128 + 128]
                return hbh[:, k * 512 + W1 + (vt - 10) * 128:k * 512 + W1 + (vt - 9) * 128]

            for vt in range(12):
                for n0, nn in ((0, 512), (512, 256)):
                    pt = psA.tile([128, 512], dt.float32, tag="work")
                    for k in range(HT):
                        nc.tensor.matmul(
                            pt[:, :nn], v_stat(vt, k), wvt[k][:, n0:n0 + nn],
                            start=(k == 0), stop=(k == HT - 1))
                    dst = v_ap(vt).rearrange("p (n c) -> p n c", c=VH)
                    h0, nh_ = n0 // HD, nn // HD
                    src = pt[:, :nn].rearrange("p (n c) -> p n c", c=HD)
                    nc.scalar.activation(dst[:, h0:h0 + nh_, 0:HD], src[:], AF.Copy)

            # --- attention per head ---
            att_cm = tc.tile_pool(name="attps", bufs=2, space="PSUM")
            att_ps = att_cm.__enter__()
            for hd_i in range(NH):
                ft, r0 = hd_i // 2, (hd_i % 2) * HD
                for qt in range(2):
                    pb = pbp.tile([128, 8 * 512], dt.bfloat16, tag="pb")
                    for j in range(8):
                        kt = 4 * qt + j
                        sc = att_ps.tile([128, 512], dt.float32, tag="sc")
                        nc.tensor.matmul(
                            sc[:], k_ap(ft, kt)[r0:r0 + HD, :],
                            qb[r0:r0 + HD, hs(ft, qt)],
                            start=True, stop=True)
                        pj = pb[:, j * 512:(j + 1) * 512]
                        nc.vector.tensor_add(
                            pj, sc[:], msk[:, (qt * 8 + j) * 512:(qt * 8 + j + 1) * 512])
                    nc.scalar.activation(pb[:], pb[:], AF.Exp, scale=ISQ)
                    pv = att_ps.tile([VH, 512], dt.float32, tag="pv")
                    for j in range(8):
                        nc.tensor.matmul(
                            pv[:], v_ap(4 * qt + j)[:, hd_i * VH:(hd_i + 1) * VH],
                            pb[:, j * 512:(j + 1) * 512],
                            start=(j == 0), stop=(j == 7))
                    r128 = rp.tile([128, 1024], dt.float32, tag="r128")
                    rb128 = rp.tile([128, 512], dt.bfloat16, tag="rb128")
                    nc.scalar.activation(r128[0:1, 0:512], pv[HD:VH, :], AF.Copy)
                    nc.vector.reciprocal_approx_fast(r128[0:1, 512:1024],
                                                     r128[0:1, 0:512])
                    nc.vector.tensor_copy(rb128[0:1, :], r128[0:1, 512:1024])
                    bc = att_ps.tile([HD, 512], dt.float32, tag="bcr")
                    nc.tensor.matmul(bc[:], ones_r[0:1, 0:HD], rb128[0:1, :],
                                     start=True, stop=True)
                    bcs = rp.tile([HD, 512], dt.bfloat16, tag="otmp", name="bcs")
                    nc.scalar.activation(bcs[:], bc[:], AF.Copy)
                    ot = rp.tile([HD, 512], dt.bfloat16, tag="otmp")
                    nc.vector.tensor_mul(ot[:], pv[0:HD, :], bcs[:])
                    nc.scalar.activation(
                        ob[r0:r0 + HD, hs(ft, qt)], ot[:], AF.Identity,
                        bias=vcol(2, ft)[r0:r0 + HD, :])

            att_cm.__exit__(None, None, None)

            # --- O projection + residual + LN1 ---
            def o_out(mt, qt, pt):
                sl = hs(mt, qt)
                nc.vector.scalar_tensor_tensor(h[:, sl], pt[:], vcol(3, mt), h[:, sl],
                                               OP.add, OP.add)
            w_proj(Wo[l], lambda k, qt: ob[:, hs(k, qt)], o_out)
            with tc.tile_pool(name="lnps", bufs=1, space="PSUM") as lp1:
                layernorm(lambda ft: vcol(4, ft), lambda ft: vcol(5, ft), lp1)

            # --- FFN ---
            ffn_cm = tc.tile_pool(name="ffps", bufs=1, space="PSUM")
            ffn_ps = ffn_cm.__enter__()
            for qt in range(2):
                fpt = [ffn_ps.tile([128, 512], dt.float32, tag=f"ff2_{m}", name=f"ff2_{m}") for m in range(HT)]
                for ch in range(6):  # 6 chunks of 4 mid tiles (512 cols of FF)
                    wi_ch = []
                    for k in range(HT):
                        t = wp.tile([128, 512], dt.bfloat16, tag="wi")
                        nc.sync.dma_start(
                            out=t[:],
                            in_=Wi[l, k * 128:(k + 1) * 128, ch * 512:(ch + 1) * 512])
                        wi_ch.append(t)
                    fmid = fp.tile([128, 4 * 512], dt.bfloat16, tag="fmid")
                    for mi in range(4):
                        mt = ch * 4 + mi
                        pt = psA.tile([128, 512], dt.float32, tag="work")
                        for k in range(HT):
                            nc.tensor.matmul(
                                pt[:], wi_ch[k][:, mi * 128:(mi + 1) * 128],
                                hb[:, hs(k, qt)],
                                start=(k == 0), stop=(k == HT - 1))
                        nc.scalar.activation(fmid[:, mi * 512:(mi + 1) * 512], pt[:],
                                             AF.Gelu, bias=bi_t[:, mt:mt + 1])
                    wo2_ch = []
                    for mi in range(4):
                        t = wp.tile([128, H], dt.bfloat16, tag="w768")
                        nc.sync.dma_start(
                            out=t[:],
                            in_=Wo2[l, (ch * 4 + mi) * 128:(ch * 4 + mi + 1) * 128, :])
                        wo2_ch.append(t)
                    for m in range(HT):
                        for mi in range(4):
                            kt = ch * 4 + mi
                            nc.tensor.matmul(
                                fpt[m][:], wo2_ch[mi][:, m * 128:(m + 1) * 128],
                                fmid[:, mi * 512:(mi + 1) * 512],
                                start=(kt == 0), stop=(kt == FT - 1))
                for m in range(HT):
                    sl = hs(m, qt)
                    nc.vector.scalar_tensor_tensor(h[:, sl], fpt[m][:], vcol(6, m),
                                                   h[:, sl], OP.add, OP.add)
            ffn_cm.__exit__(None, None, None)
            with tc.tile_pool(name="lnps", bufs=1, space="PSUM") as lp2:
                layernorm(lambda ft: vcol(7, ft), lambda ft: vcol(8, ft), lp2)

        # ---------------- output projection ----------------
        t2 = sp.tile([128, HT * T], dt.bfloat16, tag="scr")

        def d1_out(mt, qt, pt):
            nc.scalar.activation(t2[:, hs(mt, qt)], pt[:], AF.Tanh,
                                 bias=gv[:, 4 * HT + mt:4 * HT + mt + 1])
        w_proj(dn_w1, lambda k, qt: hb[:, hs(k, qt)], d1_out)

        w2t = wp.tile([128, HT * C], dt.bfloat16, tag="w768")
        for k in range(HT):
            nc.sync.dma_start(out=w2t[:, k * C:(k + 1) * C],
                              in_=dn_w2[k * 128:(k + 1) * 128, :])
        for qt in range(2):
            pt = psA.tile([128, 512], dt.float32, tag="work")
            for k in range(HT):
                nc.tensor.matmul(pt[:], w2t[:, k * C:(k + 1) * C],
                                 t2[:, hs(k, qt)], start=(k == 0), stop=(k == HT - 1))
            yo = rp.tile([128, 512], dt.float32, tag="r128", name="yout")
            nc.scalar.activation(yo[:], pt[:], AF.Identity, bias=dnb2[:])
            nc.sync.dma_start(out=y[:, qt * 512:qt * 512 + 512], in_=yo[:])

    nc.compile()
    return nc


def _host_prep(inputs, n_layers):
    f32 = np.float32
    x = np.asarray(inputs["x"], f32)
    ts = np.asarray(inputs["timesteps"])
    half = C // 2
    freqs = np.exp(-np.log(10000.0) * np.arange(half, dtype=f32) / half)
    a = ts.astype(f32)[:, None] * freqs[None, :]
    emb0 = np.concatenate([np.cos(a), np.sin(a)], -1).astype(f32)
    t1 = emb0 @ np.asarray(inputs["t_w1"], f32) + np.asarray(inputs["t_b1"], f32)
    t1 = t1 / (1.0 + np.exp(-t1))
    emb = (t1 @ np.asarray(inputs["t_w2"], f32) + np.asarray(inputs["t_b2"], f32)).astype(f32)

    def cvt(w):
        return np.ascontiguousarray(np.asarray(w, f32).astype(bf16))

    def packvec(v, nt):
        return np.ascontiguousarray(np.asarray(v, f32).reshape(nt, 128).T)

    com = dict(
        up_w1=cvt(inputs["up_w1"]), up_w2=cvt(inputs["up_w2"]),
        dn_w1=cvt(inputs["down_w1"]), dn_w2=cvt(inputs["down_w2"]),
        Wq=cvt(inputs["Wq"][:n_layers]), Wk=cvt(inputs["Wk"][:n_layers]),
        Wv=cvt(inputs["Wv"][:n_layers]), Wo=cvt(inputs["Wo"][:n_layers]),
        Wi=cvt(inputs["Wi"][:n_layers]), Wo2=cvt(inputs["Wo2"][:n_layers]),
        dn_b2=np.ascontiguousarray(np.asarray(inputs["down_b2"], f32).reshape(1, C).T),
    )
    vecs = np.stack([
        np.concatenate([packvec(np.asarray(inputs[k], f32)[l], HT) for k in
                        ("bq", "bk", "bv", "bo", "g1", "b1", "bo2", "g2", "b2", "b2")],
                       axis=1)
        for l in range(n_layers)])
    com["vecs"] = np.ascontiguousarray(vecs.astype(f32))
    com["bi_all"] = np.ascontiguousarray(
        np.stack([packvec(np.asarray(inputs["bi"], f32)[l], FT)
                  for l in range(n_layers)]).astype(f32))
    com["gvec"] = np.ascontiguousarray(np.concatenate([
        packvec(inputs["up_b1"], HT), packvec(inputs["up_b2"], HT),
        packvec(inputs["ln_g"], HT), packvec(inputs["ln_b"], HT),
        packvec(inputs["down_b1"], HT), packvec(inputs["down_b1"], HT)],
        axis=1).astype(f32))

    pos = np.asarray(inputs["pos_emb"], f32)
    mk = {}
    for sh in range(2):
        base = sh * T
        m = np.empty((2, 8, 128, 512), f32)
        for qt in range(2):
            for j in range(8):
                gk = base - W1 + (4 * qt + j) * 128 + np.arange(128)[:, None]
                gq = base + qt * 512 + np.arange(512)[None, :]
                valid = (np.abs(gk - gq) <= W1) & (gk >= 0) & (gk < S)
                m[qt, j] = np.where(valid, 0.0, NEG)
        mk[sh] = np.ascontiguousarray(m.reshape(16, 128, 512).astype(bf16))

    in_maps = []
    for c in range(8):
        b, sh = c // 2, c % 2
        sl = slice(sh * T, (sh + 1) * T)
        im = dict(com)
        im["xT"] = np.ascontiguousarray(x[b, sl].T.astype(bf16))
        im["pe"] = np.ascontiguousarray((pos[sl] + emb[b][None, :]).T.astype(f32))
        im["mks"] = mk[sh]
        in_maps.append(im)
    return in_maps


def kernel(**inputs):
    from concourse.bass_utils import run_bass_kernel_spmd

    n_layers = L
    if n_layers not in _CACHED:
        _CACHED[n_layers] = _build(n_layers)
    nc = _CACHED[n_layers]
    in_maps = _host_prep(inputs, n_layers)
    trace = os.environ.get("KERNEL_TRACE", "0") == "1"
    tmpdir = os.environ.get("KERNEL_TMPDIR") or None
    res = run_bass_kernel_spmd(nc, in_maps, list(range(8)), trace=trace,
                               tmpdir=tmpdir)
    if getattr(res, "exec_time_ns", None):
        print(f"HW exec time: {res.exec_time_ns} ns")
    out = np.empty((B, S, C), np.float32)
    for c in range(8):
        b, sh = c // 2, c % 2
        out[b, sh * T:(sh + 1) * T, :] = res.results[c]["y"].T
    return out
